# revision 1
# baseline (speedup 1.0000x reference)
"""Trainium2 Bass kernel for nn_Decoder_14139032338987 (sparse_attention).

One decoder step: embedding -> 4-layer LSTM -> Gaussian local-window
attention -> output projection -> vocab logits.  B=128, H=1024, V=32000.

Distribution over 8 NeuronCores (B kept whole on every core):
  - LSTM: tensor-parallel.  Core r computes a 128-wide h-slice of all four
    gates, producing x'[:, slice_r]; slices are transposed on-chip and
    AllGathered between layers (the AG output [1024,128] is exactly the
    transposed activation the next layer's matmul needs as lhsT).
  - Attention: p-chain replicated (needs full `out`), window gather and
    softmax sharded over B (16 rows/core) with (l,b)-packed partition
    layout; ctx re-assembled with a 0/1 selector matmul, AllGathered.
  - o2 projection replicated; vocab projection sharded over V (4000/core).
Host does layout only: embedding row gather, weight transposes/casts,
slicing, and final concat of the per-core logit slices.

SPMD note: the program is identical on all cores; every core-dependent
access (this core's 16 batch rows) goes through indirect-DMA gathers whose
index tensors are per-core host constants.
"""

import numpy as np

H = 1024
V = 32000
NL = 4
W = 10
B = 128
T = 532
L = 2 * W + 1  # 21
HALF = 512
STD2 = (W / 2.0) ** 2  # 25.0
NC = 8
HS = H // NC     # 128 h-slice per core
BS = B // NC     # 16 batch rows per core
VS = V // NC     # 4000 vocab rows per core
NG = (L * BS + 127) // 128  # 3 gather groups of (l,b) rows
ROWS = L * BS  # 336

_CACHE = {}


def _build(S_val: float):
    import concourse.bass as bass
    import concourse.mybir as mybir
    import concourse.bacc as bacc
    import concourse.tile as tile

    dt = mybir.dt
    f32, bf16, i32 = dt.float32, dt.bfloat16, dt.int32
    AF = mybir.ActivationFunctionType
    OP = mybir.AluOpType
    AP = bass.AP
    IOA = bass.IndirectOffsetOnAxis

    nc = bacc.Bacc("TRN2", target_bir_lowering=False, debug=False,
                   enable_asserts=False, num_devices=NC)

    def din(name, shape, d):
        return nc.dram_tensor(name, shape, d, kind="ExternalInput").ap()

    # ---- inputs (per-core data supplied via in_maps) ----
    x0T = din("x0T", [H, B], bf16)
    h0T = din("h0T", [NL * H, B], bf16)
    c0s = din("c0s", [B, NL * HS], f32)
    wT = din("wT", [NL * 2 * H, 512], bf16)        # (l, src, k) tiles
    gbias = din("gbias", [1, NL * 512], f32)
    aW1T = din("aW1T", [H, HALF], f32)
    aW2r = din("aW2r", [128, HALF], f32)
    ab2 = din("ab2", [128, 1], f32)
    dW1T = din("dW1T", [2 * H, H], bf16)
    db1r = din("db1r", [1, H], f32)
    ab1r = din("ab1r", [1, HALF], f32)
    dW2T = din("dW2T", [H, VS], bf16)
    db2r = din("db2r", [1, VS], f32)
    encs = din("encs", [T * BS, H], f32)
    identb = din("identb", [128, 128], bf16)
    identf = din("identf", [128, 128], f32)
    iotaL = din("iotaL", [BS, L], f32)
    iotaRow = din("iotaRow", [128, NG], f32)
    clampR = din("clampR", [128, 1], f32)
    repMc = din("repMc", [128, 128], f32)
    selMfc = din("selMfc", [128, BS], f32)
    maskCc = din("maskCc", [128, 24], f32)
    lmapc = din("lmapc", [24, NG * 128], f32)
    selM = din("selM", [128, BS], bf16)
    idxc = din("idxc", [128, 2], i32)   # col0: r*16+p%16 ; col1[0:16]: r*16+p

    yout = nc.dram_tensor("yout", [B, VS], f32, kind="ExternalOutput").ap()

    RG = [list(range(NC))]

    with tile.TileContext(nc) as tc:
        with tc.tile_pool(name="const", bufs=1) as cp, \
             tc.tile_pool(name="dw2p", bufs=1) as dw2p, \
             tc.tile_pool(name="dram", bufs=1, space="DRAM") as dp, \
             tc.tile_pool(name="work", bufs=1) as wk, \
             tc.tile_pool(name="ps_tr", bufs=2, space="PSUM") as ps_tr:

            # ---- persistent weight loads (start immediately, overlap all) ----
            dw2_sb = dw2p.tile([128, 8 * VS], bf16)
            for k in range(8):
                nc.sync.dma_start(out=dw2_sb[:, k * VS:(k + 1) * VS],
                                  in_=dW2T[k * 128:(k + 1) * 128, :])
            aw1_sb = cp.tile([128, 8 * HALF], f32)
            for k in range(8):
                nc.sync.dma_start(out=aw1_sb[:, k * HALF:(k + 1) * HALF],
                                  in_=aW1T[k * 128:(k + 1) * 128, :])
            idb = cp.tile([128, 128], bf16)
            nc.sync.dma_start(out=idb[:], in_=identb[:])
            idf = cp.tile([128, 128], f32)
            nc.sync.dma_start(out=idf[:], in_=identf[:])
            selM_sb = cp.tile([128, BS], bf16)
            nc.sync.dma_start(out=selM_sb[:], in_=selM[:])
            iotaL_sb = cp.tile([BS, L], f32)
            nc.sync.dma_start(out=iotaL_sb[:], in_=iotaL[:])
            ab1_sb = cp.tile([1, HALF], f32)
            nc.sync.dma_start(out=ab1_sb[:], in_=ab1r[:])
            ones_f = cp.tile([1, 128], f32)
            nc.vector.memset(ones_f[:], 1.0)
            iotaR_sb = cp.tile([128, NG], f32)
            nc.sync.dma_start(out=iotaR_sb[:], in_=iotaRow[:])
            clamp_sb = cp.tile([128, 1], f32)
            nc.sync.dma_start(out=clamp_sb[:], in_=clampR[:])
            repM_sb = cp.tile([128, 128], f32)
            nc.sync.dma_start(out=repM_sb[:], in_=repMc[:])
            selMf_sb = cp.tile([128, BS], f32)
            nc.sync.dma_start(out=selMf_sb[:], in_=selMfc[:])
            maskC_sb = cp.tile([128, 24], f32)
            nc.sync.dma_start(out=maskC_sb[:], in_=maskCc[:])
            lmap_sb = cp.tile([24, NG * 128], f32)
            nc.sync.dma_start(out=lmap_sb[:], in_=lmapc[:])
            idxc_sb = cp.tile([128, 2], i32)
            nc.sync.dma_start(out=idxc_sb[:], in_=idxc[:])
            gb_sb = cp.tile([1, NL * 512], f32)
            nc.sync.dma_start(out=gb_sb[:], in_=gbias[:])
            aw2_sb = cp.tile([128, HALF], f32)
            nc.sync.dma_start(out=aw2_sb[:], in_=aW2r[:])
            ab2_sb = cp.tile([128, 1], f32)
            nc.sync.dma_start(out=ab2_sb[:], in_=ab2[:])

            def pbc(ap, n):
                # [1, F] -> [n, F] partition broadcast
                b = ap.partition_broadcast(n)
                return b.rearrange("p a f -> p (a f)") if b.ndim == 3 else b

            # ============================ LSTM ============================
            outT_sb = wk.tile([128, H], f32)      # final hidden, transposed
            outTb_sb = wk.tile([128, H], bf16)
            ago3 = dp.tile([H, 256], f32, name="ago3", addr_space="Shared")
            out_bh = dp.tile([B, H], f32, name="out_bh")

            with tc.tile_pool(name="lstmw", bufs=1) as lw, \
                 tc.tile_pool(name="lstm_work", bufs=1) as lwk, \
                 tc.tile_pool(name="ps_g", bufs=2, space="PSUM") as ps_g:
                w_sb = lw.tile([128, NL * 2 * 8 * 512], bf16)
                nc.sync.dma_start(
                    out=w_sb[:].rearrange("p (m c) -> p m c", c=512),
                    in_=wT[:].rearrange("(m p) c -> p m c", p=128))
                c0_sb = lwk.tile([B, NL * HS], f32)
                nc.sync.dma_start(out=c0_sb[:], in_=c0s[:])
                h0T_sb = lwk.tile([128, NL * 8 * 128], bf16)
                nc.sync.dma_start(
                    out=h0T_sb[:].rearrange("p (m b) -> p m b", b=128),
                    in_=h0T[:].rearrange("(m p) b -> p m b", p=128))
                xT_sb = lwk.tile([128, H], bf16, tag="xT", bufs=2)
                nc.sync.dma_start(
                    out=xT_sb[:].rearrange("p (k b) -> p k b", b=128),
                    in_=x0T[:].rearrange("(k p) b -> p k b", p=128))

                for l in range(NL):
                    g_ps = ps_g.tile([128, 512], f32, tag="g")
                    mm = 0
                    for src in range(2):
                        for k in range(8):
                            lhsT = (xT_sb[:, k * 128:(k + 1) * 128] if src == 0
                                    else h0T_sb[:, (l * 8 + k) * 128:(l * 8 + k + 1) * 128])
                            nc.tensor.matmul(
                                out=g_ps[:],
                                lhsT=lhsT,
                                rhs=w_sb[:, ((l * 2 + src) * 8 + k) * 512:
                                         ((l * 2 + src) * 8 + k + 1) * 512],
                                start=(mm == 0), stop=False)
                            mm += 1
                    nc.tensor.matmul(out=g_ps[:], lhsT=ones_f[:],
                                     rhs=gb_sb[:, l * 512:(l + 1) * 512],
                                     start=False, stop=True)
                    i_s = lwk.tile([128, 128], f32, tag="i_s")
                    f_s = lwk.tile([128, 128], f32, tag="f_s")
                    g_t = lwk.tile([128, 128], f32, tag="g_t")
                    o_s = lwk.tile([128, 128], f32, tag="o_s")
                    nc.scalar.activation(out=i_s[:], in_=g_ps[:, 0:128], func=AF.Sigmoid)
                    nc.scalar.activation(out=f_s[:], in_=g_ps[:, 128:256], func=AF.Sigmoid)
                    nc.scalar.activation(out=g_t[:], in_=g_ps[:, 256:384], func=AF.Tanh)
                    nc.scalar.activation(out=o_s[:], in_=g_ps[:, 384:512], func=AF.Sigmoid)
                    cnew = lwk.tile([128, 128], f32, tag="cnew")
                    nc.vector.tensor_mul(out=cnew[:], in0=f_s[:],
                                         in1=c0_sb[:, l * HS:(l + 1) * HS])
                    ig = lwk.tile([128, 128], f32, tag="ig")
                    nc.vector.tensor_mul(out=ig[:], in0=i_s[:], in1=g_t[:])
                    nc.vector.tensor_add(out=cnew[:], in0=cnew[:], in1=ig[:])
                    tc_t = lwk.tile([128, 128], f32, tag="tc_t")
                    nc.scalar.activation(out=tc_t[:], in_=cnew[:], func=AF.Tanh)
                    xf = lwk.tile([128, 128], f32, tag="xf")
                    nc.vector.tensor_mul(out=xf[:], in0=o_s[:], in1=tc_t[:])

                    if l < NL - 1:
                        xb = lwk.tile([128, 128], bf16, tag="xb")
                        nc.vector.tensor_copy(out=xb[:], in_=xf[:])
                        tr_ps = ps_g.tile([128, 128], bf16, tag="tr")
                        nc.tensor.transpose(out=tr_ps[:], in_=xb[:], identity=idb[:])
                        xTs = lwk.tile([128, 128], bf16, tag="xTs")
                        nc.vector.tensor_copy(out=xTs[:], in_=tr_ps[:])
                        agi = dp.tile([128, 128], bf16, name=f"agi{l}", tag=f"agi{l}")
                        nc.sync.dma_start(out=agi[:], in_=xTs[:])
                        ago = dp.tile([H, 128], bf16, name=f"ago{l}", tag=f"ago{l}",
                                      addr_space="Shared")
                        nc.gpsimd.collective_compute(
                            "AllGather", OP.bypass, replica_groups=RG,
                            ins=[agi[:]], outs=[ago[:]])
                        xT_sb = lwk.tile([128, H], bf16, tag="xT", bufs=2)
                        nc.sync.dma_start(
                            out=xT_sb[:].rearrange("p (k b) -> p k b", b=128),
                            in_=ago[:].rearrange("(k p) b -> p k b", p=128))
                    else:
                        tr_ps = ps_g.tile([128, 128], f32, tag="tr")
                        nc.tensor.transpose(out=tr_ps[:], in_=xf[:], identity=idf[:])
                        pk = lwk.tile([128, 256], f32, tag="pk")
                        nc.vector.tensor_copy(out=pk[:, 0:128], in_=tr_ps[:])
                        nc.vector.tensor_copy(out=pk[:, 128:256], in_=xf[:])
                        agi3 = dp.tile([128, 256], f32, name="agi3")
                        nc.sync.dma_start(out=agi3[:], in_=pk[:])
                        nc.gpsimd.collective_compute(
                            "AllGather", OP.bypass, replica_groups=RG,
                            ins=[agi3[:]], outs=[ago3[:]])
                        nc.sync.dma_start(
                            out=outT_sb[:].rearrange("p (k b) -> p k b", b=128),
                            in_=ago3[:, 0:128].rearrange("(k p) b -> p k b", p=128))
                        nc.vector.tensor_copy(out=outTb_sb[:], in_=outT_sb[:])
                        # reshuffle to out[b, h] layout in DRAM for the
                        # attention score broadcast (local DMA, no core offset)
                        nc.sync.dma_start(
                            out=out_bh[:].rearrange("b (k f) -> b k f", f=128),
                            in_=AP(ago3[:].tensor, 128,
                                   [[256, 128], [128 * 256, 8], [1, 128]]))

            # ============================ p-chain ============================
            with tc.tile_pool(name="att", bufs=1) as at:
              with tc.tile_pool(name="ps_a", bufs=1, space="PSUM") as ps_a:
                pt_ps = ps_a.tile([128, HALF], f32, tag="pt")
                for k in range(8):
                    nc.tensor.matmul(out=pt_ps[:],
                                     lhsT=outT_sb[:, k * 128:(k + 1) * 128],
                                     rhs=aw1_sb[:, k * HALF:(k + 1) * HALF],
                                     start=(k == 0), stop=False)
                nc.tensor.matmul(out=pt_ps[:], lhsT=ones_f[:], rhs=ab1_sb[:],
                                 start=False, stop=True)
                pt = at.tile([128, HALF], f32)
                nc.scalar.activation(out=pt[:], in_=pt_ps[:], func=AF.Tanh)
                scr5 = at.tile([128, HALF], f32)
                z = at.tile([128, 1], f32)
                nc.vector.scalar_tensor_tensor(
                    out=scr5[:], in0=pt[:], scalar=1.0,
                    in1=aw2_sb[:], op0=OP.mult, op1=OP.mult,
                    accum_out=z[:])
                sg = at.tile([128, 1], f32)
                nc.scalar.activation(out=sg[:], in_=z[:], func=AF.Sigmoid,
                                     bias=ab2_sb[:])
                s_f = at.tile([128, 1], f32)       # p - W  (pre-round)
                nc.vector.tensor_scalar_mul(out=s_f[:], in0=sg[:], scalar1=float(S_val))
                r_ = at.tile([128, 1], f32)
                nc.vector.tensor_scalar_add(out=r_[:], in0=s_f[:], scalar1=0.5)
                # floor(r_) robust to the f32->i32 cast rounding mode:
                # f = cast(r_); if f > r_ then f -= 1
                ti = at.tile([128, 1], i32)
                nc.vector.tensor_copy(out=ti[:], in_=r_[:])
                tf = at.tile([128, 1], f32)
                nc.vector.tensor_copy(out=tf[:], in_=ti[:])
                cond = at.tile([128, 1], f32)
                nc.vector.tensor_tensor(out=cond[:], in0=tf[:], in1=r_[:],
                                        op=OP.is_gt)
                stf = at.tile([128, 1], f32)       # start (rounded, float)
                nc.vector.tensor_sub(out=stf[:], in0=tf[:], in1=cond[:])
                sti = at.tile([128, 1], i32)
                nc.vector.tensor_copy(out=sti[:], in_=stf[:])
                d0 = at.tile([128, 1], f32)        # start - p  (= stf - s_f - W)
                nc.vector.tensor_sub(out=d0[:], in0=stf[:], in1=s_f[:])
                nc.vector.tensor_scalar_add(out=d0[:], in0=d0[:], scalar1=-float(W))

                # ---- cross-partition replication via 0/1 matmuls (no DMA) ----
                # pk2: [start_f | start-p] per global-b partition
                pk2 = at.tile([128, 2], f32)
                nc.vector.tensor_copy(out=pk2[:, 0:1], in_=stf[:])
                nc.vector.tensor_copy(out=pk2[:, 1:2], in_=d0[:])
                # stf16/d016 for this core's 16 batch rows:
                g16_ps = ps_a.tile([BS, 2], f32, tag="scr", bufs=1)
                nc.tensor.matmul(out=g16_ps[:], lhsT=repM_sb[:, 0:BS],
                                 rhs=pk2[:], start=True, stop=True)
                g16 = at.tile([BS, 2], f32)
                nc.vector.tensor_copy(out=g16[:], in_=g16_ps[:])
                stf16 = g16[:, 0:1]
                d016 = g16[:, 1:2]
                # start_f replicated to all (l,b) rows:
                str_ps = ps_a.tile([128, 1], f32, tag="scr", bufs=1)
                nc.tensor.matmul(out=str_ps[:], lhsT=repM_sb[:],
                                 rhs=stf[:], start=True, stop=True)
                stf_rep = at.tile([128, 1], f32)
                nc.vector.tensor_copy(out=stf_rep[:], in_=str_ps[:])
                orep = at.tile([128, H], f32)
                nc.gpsimd.indirect_dma_start(
                    out=orep[:], out_offset=None, in_=out_bh[:],
                    in_offset=IOA(ap=idxc_sb[:, 0:1], axis=0))

                # ==================== gather + score ====================
                sel = [at.tile([128, H], f32, name=f"sel{g}", tag=f"sel{g}")
                       for g in range(NG)]
                sc_col = at.tile([128, NG], f32)
                nc.vector.memset(sc_col[:], 0.0)
                cnt = [128, 128, ROWS - 256]
                for g in range(NG):
                    idxf = at.tile([128, 1], f32, tag="idxf", bufs=3)
                    nc.vector.tensor_scalar_mul(out=idxf[:], in0=stf_rep[:],
                                                scalar1=float(BS))
                    nc.vector.tensor_add(out=idxf[:], in0=idxf[:],
                                         in1=iotaR_sb[:, g:g + 1])
                    nc.vector.tensor_tensor(out=idxf[:], in0=idxf[:],
                                            in1=clamp_sb[:], op=OP.min)
                    idx = at.tile([128, 1], i32, tag="idx", bufs=3)
                    nc.vector.tensor_copy(out=idx[:], in_=idxf[:])
                    nc.gpsimd.indirect_dma_start(
                        out=sel[g][0:cnt[g], :], out_offset=None,
                        in_=encs[:],
                        in_offset=IOA(ap=idx[0:cnt[g], :1], axis=0))
                    scrH = at.tile([128, H], f32, tag="scrH", bufs=1)
                    nc.vector.scalar_tensor_tensor(
                        out=scrH[0:cnt[g], :], in0=orep[0:cnt[g], :], scalar=1.0,
                        in1=sel[g][0:cnt[g], :], op0=OP.mult, op1=OP.mult,
                        accum_out=sc_col[0:cnt[g], g:g + 1])

                # -------- [16, 21] softmax block --------
                # sc16[b, l] = sc_col[(l%8)*16+b, l//8] via a selector matmul:
                # X[p, c] = sc_col[p, c//8] * maskC[p, c]; sc24 = selMf.T @ X
                X = at.tile([128, 24], f32)
                nc.vector.tensor_tensor(
                    out=X[:].rearrange("p (g li) -> p g li", g=NG),
                    in0=sc_col[:].unsqueeze(2).broadcast_to([128, NG, 8]),
                    in1=maskC_sb[:].rearrange("p (g li) -> p g li", g=NG),
                    op=OP.mult)
                sc_ps = ps_a.tile([BS, 24], f32, tag="scr", bufs=1)
                nc.tensor.matmul(out=sc_ps[:], lhsT=selMf_sb[:], rhs=X[:],
                                 start=True, stop=True)
                sc24 = at.tile([BS, 24], f32)
                nc.vector.tensor_copy(out=sc24[:], in_=sc_ps[:])
                sc16 = sc24[:, 0:L]

                pos = at.tile([BS, L], f32)
                nc.vector.tensor_scalar(out=pos[:], in0=iotaL_sb[:],
                                        scalar1=stf16, scalar2=None, op0=OP.add)
                v1 = at.tile([BS, L], f32)
                nc.vector.tensor_scalar(out=v1[:], in0=pos[:], scalar1=float(W),
                                        scalar2=None, op0=OP.is_ge)
                v2 = at.tile([BS, L], f32)
                nc.vector.tensor_scalar(out=v2[:], in0=pos[:],
                                        scalar1=float(S_val + W),
                                        scalar2=None, op0=OP.is_lt)
                nc.vector.tensor_mul(out=v1[:], in0=v1[:], in1=v2[:])
                sm = at.tile([BS, L], f32)
                nc.vector.tensor_scalar_add(out=sm[:], in0=sc16, scalar1=-1e-12)
                nc.vector.tensor_mul(out=sm[:], in0=sm[:], in1=v1[:])
                nc.vector.tensor_scalar_add(out=sm[:], in0=sm[:], scalar1=1e-12)
                mx = at.tile([BS, 1], f32)
                nc.vector.tensor_reduce(out=mx[:], in_=sm[:],
                                        axis=mybir.AxisListType.X, op=OP.max)
                nmx = at.tile([BS, 1], f32)
                nc.vector.tensor_scalar_mul(out=nmx[:], in0=mx[:], scalar1=-1.0)
                ex = at.tile([BS, L], f32)
                se = at.tile([BS, 1], f32)
                nc.scalar.activation(out=ex[:], in_=sm[:], func=AF.Exp,
                                     bias=nmx[:], accum_out=se[:])
                ri = at.tile([BS, 1], f32)
                nc.vector.reciprocal(out=ri[:], in_=se[:])
                aa = at.tile([BS, L], f32)
                nc.vector.tensor_scalar(out=aa[:], in0=ex[:], scalar1=ri[:],
                                        scalar2=None, op0=OP.mult)
                # gauss: pos - p = l + (start - p) = l + d016
                dd = at.tile([BS, L], f32)
                nc.vector.tensor_scalar(out=dd[:], in0=iotaL_sb[:],
                                        scalar1=d016, scalar2=None, op0=OP.add)
                d2 = at.tile([BS, L], f32)
                nc.vector.tensor_mul(out=d2[:], in0=dd[:], in1=dd[:])
                gs = at.tile([BS, L], f32)
                nc.scalar.activation(out=gs[:], in_=d2[:], func=AF.Exp,
                                     scale=-1.0 / (2.0 * STD2))
                nc.vector.tensor_mul(out=aa[:], in0=aa[:], in1=gs[:])
                # relayout a -> a-weighted selector Sa via PE:
                # aaT = aa.T (PE transpose), Sa_g = (Lmap_g.T @ aaT) * selMf
                aa24 = at.tile([BS, 24], f32)
                nc.vector.memset(aa24[:], 0.0)
                nc.vector.tensor_copy(out=aa24[:, 0:L], in_=aa[:])
                aaT_ps = ps_a.tile([24, BS], f32, tag="scr", bufs=1)
                nc.tensor.transpose(out=aaT_ps[:], in_=aa24[:],
                                    identity=idf[0:BS, 0:BS])
                aaT = at.tile([24, BS], f32)
                nc.vector.tensor_copy(out=aaT[:], in_=aaT_ps[:])

                # ==================== ctx ====================
                ctx_ps = ps_a.tile([BS, H], f32, tag="ctx")
                for g in range(NG):
                    sa_ps = ps_a.tile([128, BS], f32, tag="sa", bufs=1)
                    nc.tensor.matmul(out=sa_ps[:],
                                     lhsT=lmap_sb[:, g * 128:(g + 1) * 128],
                                     rhs=aaT[:], start=True, stop=True)
                    sa = at.tile([128, BS], bf16, tag="sab", bufs=3)
                    nc.vector.tensor_mul(out=sa[:], in0=sa_ps[:],
                                         in1=selMf_sb[:])
                    scd = at.tile([128, H], bf16, tag="scd", bufs=3)
                    nc.vector.tensor_copy(out=scd[0:cnt[g], :],
                                          in_=sel[g][0:cnt[g], :])
                    for n in range(2):
                        nc.tensor.matmul(
                            out=ctx_ps[:, n * 512:(n + 1) * 512],
                            lhsT=sa[0:cnt[g], :],
                            rhs=scd[0:cnt[g], n * 512:(n + 1) * 512],
                            start=(g == 0), stop=(g == NG - 1))
                ctxb = at.tile([BS, H], bf16)
                nc.vector.tensor_copy(out=ctxb[:], in_=ctx_ps[:])
              if True:
                ctxi = dp.tile([BS, H], bf16, name="ctxi")
                nc.sync.dma_start(out=ctxi[:], in_=ctxb[:])
                ctxo = dp.tile([B, H], bf16, name="ctxo", addr_space="Shared")
                nc.gpsimd.collective_compute(
                    "AllGather", OP.bypass, replica_groups=RG,
                    ins=[ctxi[:]], outs=[ctxo[:]])

                # ==================== o2 ====================
                ctx_sb = at.tile([B, H], bf16)
                nc.sync.dma_start(out=ctx_sb[:], in_=ctxo[:])
                ctxT = at.tile([128, H], bf16)
                for k in range(8):
                    trp = ps_tr.tile([128, 128], bf16, tag="tr2")
                    nc.tensor.transpose(out=trp[:],
                                        in_=ctx_sb[:, k * 128:(k + 1) * 128],
                                        identity=idb[:])
                    nc.vector.tensor_copy(out=ctxT[:, k * 128:(k + 1) * 128],
                                          in_=trp[:])
                o2b = at.tile([128, H], bf16)
                o2T = at.tile([128, H], bf16)
                with tc.tile_pool(name="dw1p", bufs=1) as dw1p, \
                     tc.tile_pool(name="ps_o2", bufs=1, space="PSUM") as ps_o2:
                    db1_sb = dw1p.tile([1, H], f32)
                    nc.sync.dma_start(out=db1_sb[:], in_=db1r[:])
                    dw1_sb = dw1p.tile([128, 16 * H], bf16)
                    nc.sync.dma_start(
                        out=dw1_sb[:].rearrange("p (m c) -> p m c", c=H),
                        in_=dW1T[:].rearrange("(m p) c -> p m c", p=128))
                    o2_ps = ps_o2.tile([128, H], f32, tag="o2")
                    for k in range(16):
                        lhsT = (ctxT[:, k * 128:(k + 1) * 128] if k < 8
                                else outTb_sb[:, (k - 8) * 128:(k - 7) * 128])
                        for n in range(2):
                            nc.tensor.matmul(
                                out=o2_ps[:, n * 512:(n + 1) * 512],
                                lhsT=lhsT,
                                rhs=dw1_sb[:, k * H + n * 512:k * H + (n + 1) * 512],
                                start=(k == 0), stop=False)
                    for n in range(2):
                        nc.tensor.matmul(out=o2_ps[:, n * 512:(n + 1) * 512],
                                         lhsT=ones_f[:],
                                         rhs=db1_sb[:, n * 512:(n + 1) * 512],
                                         start=False, stop=(n == 1))
                    nc.scalar.activation(out=o2b[:], in_=o2_ps[:], func=AF.Tanh)
                    for k in range(8):
                        trp = ps_tr.tile([128, 128], bf16, tag="tr2")
                        nc.tensor.transpose(out=trp[:],
                                            in_=o2b[:, k * 128:(k + 1) * 128],
                                            identity=idb[:])
                        nc.vector.tensor_copy(out=o2T[:, k * 128:(k + 1) * 128],
                                              in_=trp[:])

                # ==================== vocab ====================
                with tc.tile_pool(name="ps_y", bufs=3, space="PSUM") as ps_y, \
                     tc.tile_pool(name="ysb", bufs=3) as ysb:
                    db2_sb = ysb.tile([1, VS], f32, bufs=1)
                    nc.sync.dma_start(out=db2_sb[:], in_=db2r[:])
                    nch = (VS + 511) // 512
                    for n in range(nch):
                        cw = min(512, VS - n * 512)
                        y_ps = ps_y.tile([128, 512], f32, tag="y")
                        for k in range(8):
                            nc.tensor.matmul(
                                out=y_ps[:, 0:cw],
                                lhsT=o2T[:, k * 128:(k + 1) * 128],
                                rhs=dw2_sb[:, k * VS + n * 512:k * VS + n * 512 + cw],
                                start=(k == 0), stop=False)
                        nc.tensor.matmul(
                            out=y_ps[:, 0:cw], lhsT=ones_f[:],
                            rhs=db2_sb[:, n * 512:n * 512 + cw],
                            start=False, stop=True)
                        y_sb = ysb.tile([128, 512], f32, tag="ysb")
                        nc.vector.tensor_copy(out=y_sb[:, 0:cw], in_=y_ps[:, 0:cw])
                        nc.sync.dma_start(out=yout[:, n * 512:n * 512 + cw],
                                          in_=y_sb[:, 0:cw])

    nc.compile()
    return nc


def _prep_inputs(inputs):
    """Host-side layout: returns list of per-core in_maps."""
    import ml_dtypes
    bf16 = ml_dtypes.bfloat16

    enc = np.asarray(inputs["encoder_output"], np.float32)      # [T, B, H]
    h0 = np.asarray(inputs["h0"], np.float32)
    c0 = np.asarray(inputs["c0"], np.float32)
    emb = np.asarray(inputs["emb"], np.float32)
    Wih = np.asarray(inputs["Wih"], np.float32)
    Whh = np.asarray(inputs["Whh"], np.float32)
    bih = np.asarray(inputs["bih"], np.float32)
    bhh = np.asarray(inputs["bhh"], np.float32)
    aW1 = np.asarray(inputs["aW1"], np.float32)
    aW2 = np.asarray(inputs["aW2"], np.float32)
    ab2 = np.asarray(inputs["ab2"], np.float32)
    dW1 = np.asarray(inputs["dW1"], np.float32)
    db1 = np.asarray(inputs["db1"], np.float32)
    dW2 = np.asarray(inputs["dW2"], np.float32)
    db2 = np.asarray(inputs["db2"], np.float32)
    word = np.asarray(inputs["word"]).astype(np.int64)

    x0 = emb[word[0]]                                            # [B, H]
    x0T = np.ascontiguousarray(x0.T).astype(bf16)
    h0T = np.ascontiguousarray(h0.transpose(0, 2, 1)).reshape(NL * H, B).astype(bf16)

    ident_b = np.eye(128, dtype=np.float32).astype(bf16)
    ident_f = np.eye(128, dtype=np.float32)
    iotaL = np.tile(np.arange(L, dtype=np.float32).reshape(1, L), (BS, 1))
    selMat = np.zeros((128, BS), np.float32)
    for p in range(128):
        selMat[p, p % BS] = 1.0
    selMat = selMat.astype(bf16)
    iotaRow = np.zeros((128, NG), np.float32)
    for g in range(NG):
        for p in range(128):
            r = g * 128 + p
            iotaRow[p, g] = float(r if r < ROWS else 0)
    clampR = ((T - 1) * BS + (np.arange(128) % BS)).astype(np.float32).reshape(128, 1)
    selMf = selMat_f = np.zeros((128, BS), np.float32)
    for p in range(128):
        selMat_f[p, p % BS] = 1.0
    maskC = np.zeros((128, 24), np.float32)
    for p in range(128):
        for c in range(24):
            if p // BS == c % 8:
                maskC[p, c] = 1.0
    lmap = np.zeros((24, NG * 128), np.float32)
    for g in range(NG):
        for row in range(128):
            lmap[g * 8 + row // BS, g * 128 + row] = 1.0

    dW1T = np.ascontiguousarray(dW1.T).astype(bf16)              # [2H, H]
    aW1T = np.ascontiguousarray(aW1.T)                           # [H, HALF] f32
    aW2r = np.tile(aW2.reshape(1, HALF), (128, 1)).astype(np.float32)
    ab2r = np.tile(ab2.reshape(1, 1), (128, 1)).astype(np.float32)
    db1r = db1.reshape(1, H)
    ab1r = np.asarray(inputs["ab1"], np.float32).reshape(1, HALF)

    in_maps = []
    for r in range(NC):
        hs = slice(r * HS, (r + 1) * HS)
        rows = np.concatenate([np.arange(g * H + r * HS, g * H + (r + 1) * HS)
                               for g in range(4)])
        wT_l = []
        gb = np.zeros((NL, 512), np.float32)
        for l in range(NL):
            wT_l.append(np.ascontiguousarray(Wih[l][rows, :].T))  # [H, 512]
            wT_l.append(np.ascontiguousarray(Whh[l][rows, :].T))
            gb[l] = bih[l][rows] + bhh[l][rows]
        wT = np.concatenate(wT_l, axis=0).astype(bf16)           # [NL*2*H, 512]
        c0s = np.ascontiguousarray(
            np.stack([c0[l][:, hs] for l in range(NL)], axis=1).reshape(B, NL * HS))
        bs = slice(r * BS, (r + 1) * BS)
        encs = np.ascontiguousarray(enc[:, bs, :]).reshape(T * BS, H)
        vs = slice(r * VS, (r + 1) * VS)
        dW2T = np.ascontiguousarray(dW2[vs, :].T).astype(bf16)   # [H, VS]
        db2r_c = db2[vs].reshape(1, VS)
        idxc = np.zeros((128, 2), np.int32)
        idxc[:, 0] = r * BS + (np.arange(128) % BS)
        idxc[0:BS, 1] = r * BS + np.arange(BS)
        repM = np.zeros((128, 128), np.float32)
        for m in range(128):
            repM[r * BS + (m % BS), m] = 1.0
        in_maps.append({
            "x0T": np.ascontiguousarray(x0T),
            "h0T": h0T, "c0s": c0s, "wT": wT, "gbias": gb.reshape(1, NL * 512),
            "aW1T": aW1T, "aW2r": aW2r, "ab2": ab2r,
            "dW1T": dW1T, "db1r": db1r, "ab1r": ab1r, "dW2T": dW2T, "db2r": db2r_c,
            "encs": encs, "identb": ident_b, "identf": ident_f,
            "iotaL": iotaL, "iotaRow": iotaRow, "clampR": clampR,
            "selM": selMat, "idxc": idxc, "repMc": repM,
            "selMfc": selMf, "maskCc": maskC, "lmapc": lmap,
        })
    return in_maps


def kernel(**inputs):
    from concourse import bass_utils
    S_val = float(np.asarray(inputs["S"]))
    key = ("mod", S_val)
    if key not in _CACHE:
        _CACHE[key] = _build(S_val)
    nc = _CACHE[key]
    in_maps = _prep_inputs(inputs)
    res = bass_utils.run_bass_kernel_spmd(nc, in_maps, core_ids=list(range(NC)))
    y = np.concatenate([res.results[r]["yout"] for r in range(NC)], axis=1)
    return y.reshape(1, B, V).astype(np.float32)



# revision 6
# speedup vs baseline: 1.2230x; 1.2230x over previous
"""Trainium2 Bass kernel for nn_Decoder_14139032338987 (sparse_attention).

One decoder step: embedding -> 4-layer LSTM -> Gaussian local-window
attention -> output projection -> vocab logits.  B=128, H=1024, V=32000.

Distribution over 8 NeuronCores (B kept whole on every core):
  - LSTM: tensor-parallel.  Core r computes a 128-wide h-slice of all four
    gates, producing x'[:, slice_r]; slices are transposed on-chip and
    AllGathered between layers (the AG output [1024,128] is exactly the
    transposed activation the next layer's matmul needs as lhsT).
    The h-side gate matmuls of layer l+1 (h0 is a kernel input) are issued
    into their own PSUM bank before layer l's AllGather so the PE stays
    busy during the collective; only the x-side matmuls wait for the AG.
  - Attention: p-chain replicated (needs full `out`), window gather and
    softmax sharded over B (16 rows/core) with (l,b)-packed partition
    layout; ctx re-assembled with a 0/1 selector matmul, AllGathered.
  - o2 projection replicated; vocab projection sharded over V (4000/core).
Host does layout only: embedding row gather, weight transposes/casts,
slicing, and final concat of the per-core logit slices.

DMA issue order is tuned: tiny constants (PE identities) first so every
engine queue unblocks quickly, then LSTM dependencies (x0/h0/c0 and the
per-layer weight slices), then attention weights, then dW1/dW2 which are
only needed tens of microseconds into the kernel.

SPMD note: the program is identical on all cores; every core-dependent
access (this core's 16 batch rows) goes through indirect-DMA gathers whose
index tensors are per-core host constants.
"""

import numpy as np

H = 1024
V = 32000
NL = 4
W = 10
B = 128
T = 532
L = 2 * W + 1  # 21
HALF = 512
STD2 = (W / 2.0) ** 2  # 25.0
NC = 8
HS = H // NC     # 128 h-slice per core
BS = B // NC     # 16 batch rows per core
VS = V // NC     # 4000 vocab rows per core
NG = (L * BS + 127) // 128  # 3 gather groups of (l,b) rows
ROWS = L * BS  # 336

_CACHE = {}


def _build(S_val: float):
    import concourse.bass as bass
    import concourse.mybir as mybir
    import concourse.bacc as bacc
    import concourse.tile as tile

    dt = mybir.dt
    f32, bf16, i32 = dt.float32, dt.bfloat16, dt.int32
    AF = mybir.ActivationFunctionType
    OP = mybir.AluOpType
    AP = bass.AP
    IOA = bass.IndirectOffsetOnAxis

    nc = bacc.Bacc("TRN2", target_bir_lowering=False, debug=False,
                   enable_asserts=False, num_devices=NC)

    def din(name, shape, d):
        return nc.dram_tensor(name, shape, d, kind="ExternalInput").ap()

    # ---- inputs (per-core data supplied via in_maps) ----
    x0T = din("x0T", [H, B], bf16)
    h0T = din("h0T", [NL * H, B], bf16)
    c0s = din("c0s", [B, NL * HS], f32)
    wT = din("wT", [NL * 2 * H, 512], bf16)        # (l, src, k) tiles
    gbias = din("gbias", [1, NL * 512], f32)
    aW1T = din("aW1T", [H, HALF], bf16)
    aW2r = din("aW2r", [128, HALF], f32)
    ab2 = din("ab2", [128, 1], f32)
    dW1T = din("dW1T", [2 * H, H], bf16)
    db1r = din("db1r", [1, H], f32)
    ab1r = din("ab1r", [1, HALF], f32)
    dW2T = din("dW2T", [H, VS], bf16)
    db2r = din("db2r", [1, VS], f32)
    encs = din("encs", [T * BS, H], bf16)
    identb = din("identb", [128, 128], bf16)
    identf = din("identf", [128, 128], f32)
    iotaL = din("iotaL", [BS, L], f32)
    iotaRow = din("iotaRow", [128, NG], f32)
    clampR = din("clampR", [128, 1], f32)
    repMc = din("repMc", [128, 128], f32)
    selMfc = din("selMfc", [128, BS], f32)
    maskCc = din("maskCc", [128, 24], f32)
    lmapc = din("lmapc", [24, NG * 128], f32)
    idxc = din("idxc", [128, 2], i32)   # col0: r*16+p%16 ; col1[0:16]: r*16+p

    yout = nc.dram_tensor("yout", [B, VS], f32, kind="ExternalOutput").ap()

    RG = [list(range(NC))]

    with tile.TileContext(nc) as tc:
        with tc.tile_pool(name="const", bufs=1) as cp, \
             tc.tile_pool(name="wp", bufs=1) as wp, \
             tc.tile_pool(name="dram", bufs=1, space="DRAM") as dp, \
             tc.tile_pool(name="work", bufs=1) as wk, \
             tc.tile_pool(name="ps_tr", bufs=2, space="PSUM") as ps_tr:

            # ---- 1. tiny constants: unblock the PE / engine queues fast ----
            idb = cp.tile([128, 128], bf16)
            nc.sync.dma_start(out=idb[:], in_=identb[:])
            idf = cp.tile([128, 128], f32)
            nc.sync.dma_start(out=idf[:], in_=identf[:])
            gb_sb = cp.tile([1, NL * 512], f32)
            nc.sync.dma_start(out=gb_sb[:], in_=gbias[:])
            ones_f = cp.tile([1, 128], f32)
            nc.vector.memset(ones_f[:], 1.0)

            # ---- 2. LSTM dependencies, layer-0 weights first ----
            lstm_scope = [
                tc.tile_pool(name="lstm_in", bufs=1),
                tc.tile_pool(name="lstm_work", bufs=1),
                tc.tile_pool(name="ps_g", bufs=1, space="PSUM"),
            ]
            lw = lstm_scope[0].__enter__()
            lpw = lstm_scope[1].__enter__()
            ps_g = lstm_scope[2].__enter__()
            lwk = lw
            x0T_sb = lwk.tile([128, H], bf16, tag="xT", bufs=2)
            nc.sync.dma_start(
                out=x0T_sb[:].rearrange("p (k b) -> p k b", b=128),
                in_=x0T[:].rearrange("(k p) b -> p k b", p=128))
            h0T_sb = lwk.tile([128, NL * 8 * 128], bf16)
            nc.sync.dma_start(
                out=h0T_sb[:].rearrange("p (m b) -> p m b", b=128),
                in_=h0T[:].rearrange("(m p) b -> p m b", p=128))
            c0_sb = lwk.tile([B, NL * HS], f32)
            nc.sync.dma_start(out=c0_sb[:], in_=c0s[:])
            w_sb = lw.tile([128, NL * 2 * 8 * 512], bf16)
            for l in range(NL):
                nc.sync.dma_start(
                    out=w_sb[:, l * 2 * 8 * 512:(l + 1) * 2 * 8 * 512]
                        .rearrange("p (m c) -> p m c", c=512),
                    in_=wT[l * 2 * H:(l + 1) * 2 * H, :]
                        .rearrange("(m p) c -> p m c", p=128))

            # ---- 3. attention constants ----
            iotaL_sb = cp.tile([BS, L], f32)
            nc.sync.dma_start(out=iotaL_sb[:], in_=iotaL[:])
            ab1_sb = cp.tile([1, HALF], f32)
            nc.sync.dma_start(out=ab1_sb[:], in_=ab1r[:])
            iotaR_sb = cp.tile([128, NG], f32)
            nc.sync.dma_start(out=iotaR_sb[:], in_=iotaRow[:])
            clamp_sb = cp.tile([128, 1], f32)
            nc.sync.dma_start(out=clamp_sb[:], in_=clampR[:])
            repM_sb = cp.tile([128, 128], f32)
            nc.sync.dma_start(out=repM_sb[:], in_=repMc[:])
            selMf_sb = cp.tile([128, BS], f32)
            nc.sync.dma_start(out=selMf_sb[:], in_=selMfc[:])
            maskC_sb = cp.tile([128, 24], f32)
            nc.sync.dma_start(out=maskC_sb[:], in_=maskCc[:])
            lmap_sb = cp.tile([24, NG * 128], f32)
            nc.sync.dma_start(out=lmap_sb[:], in_=lmapc[:])
            idxc_sb = cp.tile([128, 2], i32)
            nc.sync.dma_start(out=idxc_sb[:], in_=idxc[:])
            aw2_sb = cp.tile([128, HALF], f32)
            nc.sync.dma_start(out=aw2_sb[:], in_=aW2r[:])
            ab2_sb = cp.tile([128, 1], f32)
            nc.sync.dma_start(out=ab2_sb[:], in_=ab2[:])
            aw1_sb = cp.tile([128, 8 * HALF], bf16)
            for k in range(8):
                nc.sync.dma_start(out=aw1_sb[:, k * HALF:(k + 1) * HALF],
                                  in_=aW1T[k * 128:(k + 1) * 128, :])

            # ---- 4. deep weights (needed latest; issued last) ----
            db1_sb = wp.tile([1, H], f32)
            nc.sync.dma_start(out=db1_sb[:], in_=db1r[:])
            dw1_sb = wp.tile([128, 16 * H], bf16)
            nc.sync.dma_start(
                out=dw1_sb[:].rearrange("p (m c) -> p m c", c=H),
                in_=dW1T[:].rearrange("(m p) c -> p m c", p=128))

            # ============================ LSTM ============================
            outT_sb = wk.tile([128, H], bf16)     # final hidden, transposed
            ago3 = dp.tile([H, 256], bf16, name="ago3", addr_space="Shared")
            out_bh = dp.tile([B, H], bf16, name="out_bh")

            if True:
                g_ps = [ps_g.tile([128, 512], f32, tag=f"g{l}",
                                  name=f"g_ps{l}")
                        for l in range(NL)]

                def g_group(l, src, xts, first, last):
                    # 8 accumulating matmuls of one source into g_ps[l]
                    for k in range(8):
                        lhsT = (xts[:, k * 128:(k + 1) * 128] if src == 0
                                else h0T_sb[:, (l * 8 + k) * 128:
                                            (l * 8 + k + 1) * 128])
                        nc.tensor.matmul(
                            out=g_ps[l][:],
                            lhsT=lhsT,
                            rhs=w_sb[:, ((l * 2 + src) * 8 + k) * 512:
                                     ((l * 2 + src) * 8 + k + 1) * 512],
                            start=(first and k == 0), stop=(last and k == 7))

                def g_bias_h(l):
                    # bias + h-side partials for layer l (no x dependency)
                    nc.tensor.matmul(out=g_ps[l][:], lhsT=ones_f[:],
                                     rhs=gb_sb[:, l * 512:(l + 1) * 512],
                                     start=True, stop=False)
                    g_group(l, 1, None, False, False)

                # layer 0: everything is available up front
                g_bias_h(0)
                xT_sb = x0T_sb
                g_group(0, 0, xT_sb, False, True)

                for l in range(NL):
                    i_s = lpw.tile([128, 128], f32, tag="i_s")
                    f_s = lpw.tile([128, 128], f32, tag="f_s")
                    g_t = lpw.tile([128, 128], f32, tag="g_t")
                    o_s = lpw.tile([128, 128], f32, tag="o_s")
                    nc.scalar.activation(out=i_s[:], in_=g_ps[l][:, 0:128],
                                         func=AF.Sigmoid)
                    nc.scalar.activation(out=f_s[:], in_=g_ps[l][:, 128:256],
                                         func=AF.Sigmoid)
                    nc.scalar.activation(out=g_t[:], in_=g_ps[l][:, 256:384],
                                         func=AF.Tanh)
                    nc.scalar.activation(out=o_s[:], in_=g_ps[l][:, 384:512],
                                         func=AF.Sigmoid)
                    cnew = lpw.tile([128, 128], f32, tag="cnew")
                    nc.vector.tensor_mul(out=cnew[:], in0=f_s[:],
                                         in1=c0_sb[:, l * HS:(l + 1) * HS])
                    ig = lpw.tile([128, 128], f32, tag="ig")
                    nc.vector.tensor_mul(out=ig[:], in0=i_s[:], in1=g_t[:])
                    nc.vector.tensor_add(out=cnew[:], in0=cnew[:], in1=ig[:])
                    tc_t = lpw.tile([128, 128], f32, tag="tc_t")
                    nc.scalar.activation(out=tc_t[:], in_=cnew[:], func=AF.Tanh)
                    xfb = lpw.tile([128, 128], bf16, tag="xfb")
                    nc.vector.tensor_mul(out=xfb[:], in0=o_s[:], in1=tc_t[:])

                    if l < NL - 1:
                        tr_ps = ps_g.tile([128, 128], bf16, tag="tr", bufs=2)
                        nc.tensor.transpose(out=tr_ps[:], in_=xfb[:],
                                            identity=idb[:])
                        xTs = lpw.tile([128, 128], bf16, tag="xTs")
                        nc.vector.tensor_copy(out=xTs[:], in_=tr_ps[:])
                        agi = dp.tile([128, 128], bf16, name=f"agi{l}",
                                      tag=f"agi{l}")
                        nc.sync.dma_start(out=agi[:], in_=xTs[:])
                        ago = dp.tile([H, 128], bf16, name=f"ago{l}",
                                      tag=f"ago{l}", addr_space="Shared")
                        nc.gpsimd.collective_compute(
                            "AllGather", OP.bypass, replica_groups=RG,
                            ins=[agi[:]], outs=[ago[:]])
                        # h+bias partials of the NEXT layer run during the AG
                        g_bias_h(l + 1)
                        xT_sb = lwk.tile([128, H], bf16, tag="xT", bufs=2)
                        nc.sync.dma_start(
                            out=xT_sb[:].rearrange("p (k b) -> p k b", b=128),
                            in_=ago[:].rearrange("(k p) b -> p k b", p=128))
                        g_group(l + 1, 0, xT_sb, False, True)
                    else:
                        tr_ps = ps_g.tile([128, 128], bf16, tag="tr", bufs=2)
                        nc.tensor.transpose(out=tr_ps[:], in_=xfb[:],
                                            identity=idb[:])
                        pk = lpw.tile([128, 256], bf16, tag="pk")
                        nc.vector.tensor_copy(out=pk[:, 0:128], in_=tr_ps[:])
                        nc.vector.tensor_copy(out=pk[:, 128:256], in_=xfb[:])
                        agi3 = dp.tile([128, 256], bf16, name="agi3")
                        nc.sync.dma_start(out=agi3[:], in_=pk[:])
                        nc.gpsimd.collective_compute(
                            "AllGather", OP.bypass, replica_groups=RG,
                            ins=[agi3[:]], outs=[ago3[:]])
                        nc.sync.dma_start(
                            out=outT_sb[:].rearrange("p (k b) -> p k b", b=128),
                            in_=ago3[:, 0:128].rearrange("(k p) b -> p k b",
                                                         p=128))
                        # reshuffle to out[b, h] layout in DRAM for the
                        # attention score broadcast (local DMA, no core offset)
                        nc.sync.dma_start(
                            out=out_bh[:].rearrange("b (k f) -> b k f", f=128),
                            in_=AP(ago3[:].tensor, 128,
                                   [[256, 128], [128 * 256, 8], [1, 128]]))
            for _cm in reversed(lstm_scope):
                _cm.__exit__(None, None, None)

            # ============================ p-chain ============================
            with tc.tile_pool(name="att", bufs=1) as at:
              with tc.tile_pool(name="ps_a", bufs=1, space="PSUM") as ps_a:
                pt_ps = ps_a.tile([128, HALF], f32, tag="pt")
                for k in range(8):
                    nc.tensor.matmul(out=pt_ps[:],
                                     lhsT=outT_sb[:, k * 128:(k + 1) * 128],
                                     rhs=aw1_sb[:, k * HALF:(k + 1) * HALF],
                                     start=(k == 0), stop=False)
                nc.tensor.matmul(out=pt_ps[:], lhsT=ones_f[:], rhs=ab1_sb[:],
                                 start=False, stop=True)
                pt = at.tile([128, HALF], f32)
                nc.scalar.activation(out=pt[:], in_=pt_ps[:], func=AF.Tanh)
                scr5 = at.tile([128, HALF], f32)
                z = at.tile([128, 1], f32)
                nc.vector.scalar_tensor_tensor(
                    out=scr5[:], in0=pt[:], scalar=1.0,
                    in1=aw2_sb[:], op0=OP.mult, op1=OP.mult,
                    accum_out=z[:])
                sg = at.tile([128, 1], f32)
                nc.scalar.activation(out=sg[:], in_=z[:], func=AF.Sigmoid,
                                     bias=ab2_sb[:])
                s_f = at.tile([128, 1], f32)       # p - W  (pre-round)
                nc.vector.tensor_scalar_mul(out=s_f[:], in0=sg[:],
                                            scalar1=float(S_val))
                r_ = at.tile([128, 1], f32)
                nc.vector.tensor_scalar_add(out=r_[:], in0=s_f[:], scalar1=0.5)
                # floor(r_) robust to the f32->i32 cast rounding mode:
                # f = cast(r_); if f > r_ then f -= 1
                ti = at.tile([128, 1], i32)
                nc.vector.tensor_copy(out=ti[:], in_=r_[:])
                tf = at.tile([128, 1], f32)
                nc.vector.tensor_copy(out=tf[:], in_=ti[:])
                cond = at.tile([128, 1], f32)
                nc.vector.tensor_tensor(out=cond[:], in0=tf[:], in1=r_[:],
                                        op=OP.is_gt)
                stf = at.tile([128, 1], f32)       # start (rounded, float)
                nc.vector.tensor_sub(out=stf[:], in0=tf[:], in1=cond[:])
                d0 = at.tile([128, 1], f32)        # start - p  (= stf - s_f - W)
                nc.vector.tensor_sub(out=d0[:], in0=stf[:], in1=s_f[:])
                nc.vector.tensor_scalar_add(out=d0[:], in0=d0[:],
                                            scalar1=-float(W))

                # ---- cross-partition replication via 0/1 matmuls (no DMA) ----
                # pk2: [start_f | start-p] per global-b partition
                pk2 = at.tile([128, 2], f32)
                nc.vector.tensor_copy(out=pk2[:, 0:1], in_=stf[:])
                nc.vector.tensor_copy(out=pk2[:, 1:2], in_=d0[:])
                # stf16/d016 for this core's 16 batch rows:
                g16_ps = ps_a.tile([BS, 2], f32, tag="scr", bufs=1)
                nc.tensor.matmul(out=g16_ps[:], lhsT=repM_sb[:, 0:BS],
                                 rhs=pk2[:], start=True, stop=True)
                g16 = at.tile([BS, 2], f32)
                nc.vector.tensor_copy(out=g16[:], in_=g16_ps[:])
                stf16 = g16[:, 0:1]
                d016 = g16[:, 1:2]
                # start_f replicated to all (l,b) rows:
                str_ps = ps_a.tile([128, 1], f32, tag="scr", bufs=1)
                nc.tensor.matmul(out=str_ps[:], lhsT=repM_sb[:],
                                 rhs=stf[:], start=True, stop=True)
                stf_rep = at.tile([128, 1], f32)
                nc.vector.tensor_copy(out=stf_rep[:], in_=str_ps[:])
                orep = at.tile([128, H], bf16)
                nc.gpsimd.indirect_dma_start(
                    out=orep[:], out_offset=None, in_=out_bh[:],
                    in_offset=IOA(ap=idxc_sb[:, 0:1], axis=0))

                # ---- gather indices (vector queue: emit before masks) ----
                sel = [at.tile([128, H], bf16, name=f"sel{g}", tag=f"sel{g}")
                       for g in range(NG)]
                cnt = [128, 128, ROWS - 256]
                idxs = []
                for g in range(NG):
                    idxf = at.tile([128, 1], f32, tag="idxf", bufs=3)
                    nc.vector.tensor_scalar_mul(out=idxf[:], in0=stf_rep[:],
                                                scalar1=float(BS))
                    nc.vector.tensor_add(out=idxf[:], in0=idxf[:],
                                         in1=iotaR_sb[:, g:g + 1])
                    nc.vector.tensor_tensor(out=idxf[:], in0=idxf[:],
                                            in1=clamp_sb[:], op=OP.min)
                    idx = at.tile([128, 1], i32, tag="idx", bufs=3)
                    nc.vector.tensor_copy(out=idx[:], in_=idxf[:])
                    idxs.append(idx)
                    nc.gpsimd.indirect_dma_start(
                        out=sel[g][0:cnt[g], :], out_offset=None,
                        in_=encs[:],
                        in_offset=IOA(ap=idx[0:cnt[g], :1], axis=0))

                # ---- window masks + gauss weights (independent of gather) ----
                pos = at.tile([BS, L], f32)
                nc.vector.tensor_scalar(out=pos[:], in0=iotaL_sb[:],
                                        scalar1=stf16, scalar2=None, op0=OP.add)
                v1 = at.tile([BS, L], f32)
                nc.vector.tensor_scalar(out=v1[:], in0=pos[:], scalar1=float(W),
                                        scalar2=None, op0=OP.is_ge)
                v2 = at.tile([BS, L], f32)
                nc.vector.tensor_scalar(out=v2[:], in0=pos[:],
                                        scalar1=float(S_val + W),
                                        scalar2=None, op0=OP.is_lt)
                nc.vector.tensor_mul(out=v1[:], in0=v1[:], in1=v2[:])
                # gauss: pos - p = l + (start - p) = l + d016
                dd = at.tile([BS, L], f32)
                nc.vector.tensor_scalar(out=dd[:], in0=iotaL_sb[:],
                                        scalar1=d016, scalar2=None, op0=OP.add)
                d2 = at.tile([BS, L], f32)
                nc.vector.tensor_mul(out=d2[:], in0=dd[:], in1=dd[:])
                gs = at.tile([BS, L], f32)
                nc.scalar.activation(out=gs[:], in_=d2[:], func=AF.Exp,
                                     scale=-1.0 / (2.0 * STD2))

                # ---- score: out . sel per (l,b) row ----
                sc_col = at.tile([128, NG], f32)
                nc.vector.memset(sc_col[:], 0.0)
                for g in range(NG):
                    scrH = at.tile([128, H], bf16, tag="scrH", bufs=1)
                    nc.vector.scalar_tensor_tensor(
                        out=scrH[0:cnt[g], :], in0=orep[0:cnt[g], :], scalar=1.0,
                        in1=sel[g][0:cnt[g], :], op0=OP.mult, op1=OP.mult,
                        accum_out=sc_col[0:cnt[g], g:g + 1])

                # -------- [16, 21] softmax block --------
                # sc16[b, l] = sc_col[(l%8)*16+b, l//8] via a selector matmul:
                # X[p, c] = sc_col[p, c//8] * maskC[p, c]; sc24 = selMf.T @ X
                X = at.tile([128, 24], f32)
                nc.vector.tensor_tensor(
                    out=X[:].rearrange("p (g li) -> p g li", g=NG),
                    in0=sc_col[:].unsqueeze(2).broadcast_to([128, NG, 8]),
                    in1=maskC_sb[:].rearrange("p (g li) -> p g li", g=NG),
                    op=OP.mult)
                sc_ps = ps_a.tile([BS, 24], f32, tag="scr", bufs=1)
                nc.tensor.matmul(out=sc_ps[:], lhsT=selMf_sb[:], rhs=X[:],
                                 start=True, stop=True)
                sc24 = at.tile([BS, 24], f32)
                nc.vector.tensor_copy(out=sc24[:], in_=sc_ps[:])
                sc16 = sc24[:, 0:L]

                sm = at.tile([BS, L], f32)
                nc.vector.tensor_scalar_add(out=sm[:], in0=sc16, scalar1=-1e-12)
                nc.vector.tensor_mul(out=sm[:], in0=sm[:], in1=v1[:])
                nc.vector.tensor_scalar_add(out=sm[:], in0=sm[:], scalar1=1e-12)
                mx = at.tile([BS, 1], f32)
                nc.vector.tensor_reduce(out=mx[:], in_=sm[:],
                                        axis=mybir.AxisListType.X, op=OP.max)
                nmx = at.tile([BS, 1], f32)
                nc.vector.tensor_scalar_mul(out=nmx[:], in0=mx[:], scalar1=-1.0)
                ex = at.tile([BS, L], f32)
                se = at.tile([BS, 1], f32)
                nc.scalar.activation(out=ex[:], in_=sm[:], func=AF.Exp,
                                     bias=nmx[:], accum_out=se[:])
                ri = at.tile([BS, 1], f32)
                nc.vector.reciprocal(out=ri[:], in_=se[:])
                aa = at.tile([BS, L], f32)
                nc.vector.tensor_scalar(out=aa[:], in0=ex[:], scalar1=ri[:],
                                        scalar2=None, op0=OP.mult)
                nc.vector.tensor_mul(out=aa[:], in0=aa[:], in1=gs[:])
                # relayout a -> a-weighted selector Sa via PE:
                # aaT = aa.T (PE transpose), Sa_g = (Lmap_g.T @ aaT) * selMf
                aa24 = at.tile([BS, 24], f32)
                nc.vector.memset(aa24[:], 0.0)
                nc.vector.tensor_copy(out=aa24[:, 0:L], in_=aa[:])
                aaT_ps = ps_a.tile([24, BS], f32, tag="scr", bufs=1)
                nc.tensor.transpose(out=aaT_ps[:], in_=aa24[:],
                                    identity=idf[0:BS, 0:BS])
                aaT = at.tile([24, BS], f32)
                nc.vector.tensor_copy(out=aaT[:], in_=aaT_ps[:])

                # ==================== ctx ====================
                ctx_ps = ps_a.tile([BS, H], f32, tag="ctx")
                for g in range(NG):
                    sa_ps = ps_a.tile([128, BS], f32, tag="sa", bufs=1)
                    nc.tensor.matmul(out=sa_ps[:],
                                     lhsT=lmap_sb[:, g * 128:(g + 1) * 128],
                                     rhs=aaT[:], start=True, stop=True)
                    sa = at.tile([128, BS], bf16, tag="sab", bufs=3)
                    nc.vector.tensor_mul(out=sa[:], in0=sa_ps[:],
                                         in1=selMf_sb[:])
                    for n in range(2):
                        nc.tensor.matmul(
                            out=ctx_ps[:, n * 512:(n + 1) * 512],
                            lhsT=sa[0:cnt[g], :],
                            rhs=sel[g][0:cnt[g], n * 512:(n + 1) * 512],
                            start=(g == 0), stop=(g == NG - 1))
                ctxb = at.tile([BS, H], bf16)
                nc.vector.tensor_copy(out=ctxb[:], in_=ctx_ps[:])
              if True:
                ctxi = dp.tile([BS, H], bf16, name="ctxi")
                nc.sync.dma_start(out=ctxi[:], in_=ctxb[:])
                ctxo = dp.tile([B, H], bf16, name="ctxo", addr_space="Shared")
                nc.gpsimd.collective_compute(
                    "AllGather", OP.bypass, replica_groups=RG,
                    ins=[ctxi[:]], outs=[ctxo[:]])

                # ==================== o2 ====================
                ctx_sb = at.tile([B, H], bf16)
                nc.sync.dma_start(out=ctx_sb[:], in_=ctxo[:])
                ctxT = at.tile([128, H], bf16)
                for k in range(8):
                    trp = ps_tr.tile([128, 128], bf16, tag="tr2")
                    nc.tensor.transpose(out=trp[:],
                                        in_=ctx_sb[:, k * 128:(k + 1) * 128],
                                        identity=idb[:])
                    nc.vector.tensor_copy(out=ctxT[:, k * 128:(k + 1) * 128],
                                          in_=trp[:])
                o2b = at.tile([128, H], bf16)
                o2T = at.tile([128, H], bf16)
                with tc.tile_pool(name="ps_o2", bufs=1, space="PSUM") as ps_o2:
                    o2_ps = ps_o2.tile([128, H], f32, tag="o2")
                    for k in range(16):
                        lhsT = (ctxT[:, k * 128:(k + 1) * 128] if k < 8
                                else outT_sb[:, (k - 8) * 128:(k - 7) * 128])
                        for n in range(2):
                            nc.tensor.matmul(
                                out=o2_ps[:, n * 512:(n + 1) * 512],
                                lhsT=lhsT,
                                rhs=dw1_sb[:, k * H + n * 512:k * H + (n + 1) * 512],
                                start=(k == 0), stop=False)
                    for n in range(2):
                        nc.tensor.matmul(out=o2_ps[:, n * 512:(n + 1) * 512],
                                         lhsT=ones_f[:],
                                         rhs=db1_sb[:, n * 512:(n + 1) * 512],
                                         start=False, stop=(n == 1))
                    nc.scalar.activation(out=o2b[:], in_=o2_ps[:], func=AF.Tanh)
                    for k in range(8):
                        trp = ps_tr.tile([128, 128], bf16, tag="tr2")
                        nc.tensor.transpose(out=trp[:],
                                            in_=o2b[:, k * 128:(k + 1) * 128],
                                            identity=idb[:])
                        nc.vector.tensor_copy(out=o2T[:, k * 128:(k + 1) * 128],
                                              in_=trp[:])

                # ==================== vocab ====================
                with tc.tile_pool(name="ps_y", bufs=3, space="PSUM") as ps_y, \
                     tc.tile_pool(name="ysb", bufs=3) as ysb, \
                     tc.tile_pool(name="ws", bufs=3) as ws:
                    db2_sb = ysb.tile([1, VS], f32, bufs=1)
                    nc.sync.dma_start(out=db2_sb[:], in_=db2r[:])
                    nch = (VS + 511) // 512
                    for n in range(nch):
                        cw = min(512, VS - n * 512)
                        wtile = ws.tile([128, 8 * 512], bf16, tag="wt")
                        for k in range(8):
                            nc.sync.dma_start(
                                out=wtile[:, k * 512:k * 512 + cw],
                                in_=dW2T[k * 128:(k + 1) * 128,
                                         n * 512:n * 512 + cw])
                        y_ps = ps_y.tile([128, 512], f32, tag="y")
                        for k in range(8):
                            nc.tensor.matmul(
                                out=y_ps[:, 0:cw],
                                lhsT=o2T[:, k * 128:(k + 1) * 128],
                                rhs=wtile[:, k * 512:k * 512 + cw],
                                start=(k == 0), stop=False)
                        nc.tensor.matmul(
                            out=y_ps[:, 0:cw], lhsT=ones_f[:],
                            rhs=db2_sb[:, n * 512:n * 512 + cw],
                            start=False, stop=True)
                        y_sb = ysb.tile([128, 512], f32, tag="ysb")
                        nc.vector.tensor_copy(out=y_sb[:, 0:cw], in_=y_ps[:, 0:cw])
                        nc.sync.dma_start(out=yout[:, n * 512:n * 512 + cw],
                                          in_=y_sb[:, 0:cw])

    nc.compile()
    return nc


def _prep_inputs(inputs):
    """Host-side layout: returns list of per-core in_maps."""
    import ml_dtypes
    bf16 = ml_dtypes.bfloat16

    enc = np.asarray(inputs["encoder_output"], np.float32)      # [T, B, H]
    h0 = np.asarray(inputs["h0"], np.float32)
    c0 = np.asarray(inputs["c0"], np.float32)
    emb = np.asarray(inputs["emb"], np.float32)
    Wih = np.asarray(inputs["Wih"], np.float32)
    Whh = np.asarray(inputs["Whh"], np.float32)
    bih = np.asarray(inputs["bih"], np.float32)
    bhh = np.asarray(inputs["bhh"], np.float32)
    aW1 = np.asarray(inputs["aW1"], np.float32)
    aW2 = np.asarray(inputs["aW2"], np.float32)
    ab2 = np.asarray(inputs["ab2"], np.float32)
    dW1 = np.asarray(inputs["dW1"], np.float32)
    db1 = np.asarray(inputs["db1"], np.float32)
    dW2 = np.asarray(inputs["dW2"], np.float32)
    db2 = np.asarray(inputs["db2"], np.float32)
    word = np.asarray(inputs["word"]).astype(np.int64)

    x0 = emb[word[0]]                                            # [B, H]
    x0T = np.ascontiguousarray(x0.T).astype(bf16)
    h0T = np.ascontiguousarray(h0.transpose(0, 2, 1)).reshape(NL * H, B).astype(bf16)

    ident_b = np.eye(128, dtype=np.float32).astype(bf16)
    ident_f = np.eye(128, dtype=np.float32)
    iotaL = np.tile(np.arange(L, dtype=np.float32).reshape(1, L), (BS, 1))
    iotaRow = np.zeros((128, NG), np.float32)
    for g in range(NG):
        for p in range(128):
            r = g * 128 + p
            iotaRow[p, g] = float(r if r < ROWS else 0)
    clampR = ((T - 1) * BS + (np.arange(128) % BS)).astype(np.float32).reshape(128, 1)
    selMf = np.zeros((128, BS), np.float32)
    for p in range(128):
        selMf[p, p % BS] = 1.0
    maskC = np.zeros((128, 24), np.float32)
    for p in range(128):
        for c in range(24):
            if p // BS == c % 8:
                maskC[p, c] = 1.0
    lmap = np.zeros((24, NG * 128), np.float32)
    for g in range(NG):
        for row in range(128):
            lmap[g * 8 + row // BS, g * 128 + row] = 1.0

    dW1T = np.ascontiguousarray(dW1.T).astype(bf16)              # [2H, H]
    aW1T = np.ascontiguousarray(aW1.T).astype(bf16)              # [H, HALF]
    aW2r = np.tile(aW2.reshape(1, HALF), (128, 1)).astype(np.float32)
    ab2r = np.tile(ab2.reshape(1, 1), (128, 1)).astype(np.float32)
    db1r = db1.reshape(1, H)
    ab1r = np.asarray(inputs["ab1"], np.float32).reshape(1, HALF)

    in_maps = []
    for r in range(NC):
        hs = slice(r * HS, (r + 1) * HS)
        rows = np.concatenate([np.arange(g * H + r * HS, g * H + (r + 1) * HS)
                               for g in range(4)])
        wT_l = []
        gb = np.zeros((NL, 512), np.float32)
        for l in range(NL):
            wT_l.append(np.ascontiguousarray(Wih[l][rows, :].T))  # [H, 512]
            wT_l.append(np.ascontiguousarray(Whh[l][rows, :].T))
            gb[l] = bih[l][rows] + bhh[l][rows]
        wT = np.concatenate(wT_l, axis=0).astype(bf16)           # [NL*2*H, 512]
        c0s = np.ascontiguousarray(
            np.stack([c0[l][:, hs] for l in range(NL)], axis=1).reshape(B, NL * HS))
        bs = slice(r * BS, (r + 1) * BS)
        encs = np.ascontiguousarray(enc[:, bs, :]).reshape(T * BS, H).astype(bf16)
        vs = slice(r * VS, (r + 1) * VS)
        dW2T = np.ascontiguousarray(dW2[vs, :].T).astype(bf16)   # [H, VS]
        db2r_c = db2[vs].reshape(1, VS)
        idxc = np.zeros((128, 2), np.int32)
        idxc[:, 0] = r * BS + (np.arange(128) % BS)
        idxc[0:BS, 1] = r * BS + np.arange(BS)
        repM = np.zeros((128, 128), np.float32)
        for m in range(128):
            repM[r * BS + (m % BS), m] = 1.0
        in_maps.append({
            "x0T": np.ascontiguousarray(x0T),
            "h0T": h0T, "c0s": c0s, "wT": wT, "gbias": gb.reshape(1, NL * 512),
            "aW1T": aW1T, "aW2r": aW2r, "ab2": ab2r,
            "dW1T": dW1T, "db1r": db1r, "ab1r": ab1r, "dW2T": dW2T, "db2r": db2r_c,
            "encs": encs, "identb": ident_b, "identf": ident_f,
            "iotaL": iotaL, "iotaRow": iotaRow, "clampR": clampR,
            "idxc": idxc, "repMc": repM,
            "selMfc": selMf, "maskCc": maskC, "lmapc": lmap,
        })
    return in_maps


def kernel(**inputs):
    from concourse import bass_utils
    S_val = float(np.asarray(inputs["S"]))
    key = ("mod", S_val)
    if key not in _CACHE:
        _CACHE[key] = _build(S_val)
    nc = _CACHE[key]
    in_maps = _prep_inputs(inputs)
    res = bass_utils.run_bass_kernel_spmd(nc, in_maps, core_ids=list(range(NC)))
    y = np.concatenate([res.results[r]["yout"] for r in range(NC)], axis=1)
    return y.reshape(1, B, V).astype(np.float32)


# revision 7
# speedup vs baseline: 1.4453x; 1.1818x over previous
"""Trainium2 Bass kernel for nn_Decoder_14139032338987 (sparse_attention).

One decoder step: embedding -> 4-layer LSTM -> Gaussian local-window
attention -> output projection -> vocab logits.  B=128, H=1024, V=32000.

Distribution over 8 NeuronCores (B kept whole on every core):
  - LSTM: tensor-parallel.  Core r computes a 128-wide h-slice of all four
    gates (host packs the rows i,f,o,g so one Sigmoid covers 3 gates),
    producing x'[:, slice_r]; slices are transposed on-chip and
    AllGathered between layers.  The h-side gate matmuls of layer l+1
    (h0 is a kernel input) are issued into their own PSUM bank before
    layer l's AllGather so the PE stays busy during the collective.
  - Attention: p-chain replicated (needs full `out`), window gather and
    softmax sharded over B (16 rows/core) with (l,b)-packed partition
    layout; ctx re-assembled with a 0/1 selector matmul, AllGathered.
  - o2 projection tensor-parallel over its 1024 outputs (128/core), the
    transposed slice AllGathered into the vocab matmul's lhsT.
  - vocab projection sharded over V (4000/core); weight tiles are
    streamed from DRAM during the matmul instead of being preloaded.

All large weights are pre-laid on the host into the exact SBUF shape so
every big DMA is a contiguous 2D copy (cheap descriptor generation on
the issuing engine).  A dummy 32-byte AllGather right at kernel start
absorbs the one-time CCOM rendezvous/warmup into the weight-load window.

SPMD note: the program is identical on all cores; every core-dependent
access (this core's 16 batch rows) goes through indirect-DMA gathers whose
index tensors are per-core host constants.
"""

import numpy as np

H = 1024
V = 32000
NL = 4
W = 10
B = 128
T = 532
L = 2 * W + 1  # 21
HALF = 512
STD2 = (W / 2.0) ** 2  # 25.0
NC = 8
HS = H // NC     # 128 h-slice per core
BS = B // NC     # 16 batch rows per core
VS = V // NC     # 4000 vocab rows per core
NG = (L * BS + 127) // 128  # 3 gather groups of (l,b) rows
ROWS = L * BS  # 336
NCH = (VS + 511) // 512     # vocab column chunks per core

_CACHE = {}


def _build(S_val: float):
    import concourse.bass as bass
    import concourse.mybir as mybir
    import concourse.bacc as bacc
    import concourse.tile as tile

    dt = mybir.dt
    f32, bf16, i32 = dt.float32, dt.bfloat16, dt.int32
    AF = mybir.ActivationFunctionType
    OP = mybir.AluOpType
    AP = bass.AP
    IOA = bass.IndirectOffsetOnAxis

    nc = bacc.Bacc("TRN2", target_bir_lowering=False, debug=False,
                   enable_asserts=False, num_devices=NC)

    def din(name, shape, d):
        return nc.dram_tensor(name, shape, d, kind="ExternalInput").ap()

    # ---- inputs (per-core data supplied via in_maps; all big weights
    #      pre-laid host-side into the SBUF [128, ...] layout) ----
    x0Tr = din("x0Tr", [128, 8 * B], bf16)
    h0Tr = din("h0Tr", [128, NL * 8 * B], bf16)
    c0s = din("c0s", [B, NL * HS], f32)
    wTr = din("wTr", [128, NL * 2 * 8 * 512], bf16)
    gbias = din("gbias", [1, NL * 512], f32)
    aW1r = din("aW1r", [128, 8 * HALF], bf16)
    aW2r = din("aW2r", [128, HALF], f32)
    ab2 = din("ab2", [128, 1], f32)
    dW1r = din("dW1r", [128, 16 * HS], bf16)       # this core's o2 slice
    db1r = din("db1r", [1, HS], f32)
    ab1r = din("ab1r", [1, HALF], f32)
    dW2s = din("dW2s", [128, NCH * 8 * 512], bf16)  # (n, k)-tiled stream
    db2r = din("db2r", [1, VS], f32)
    encs = din("encs", [T * BS, H], bf16)
    identb = din("identb", [128, 128], bf16)
    identf = din("identf", [128, 128], f32)
    iotaL = din("iotaL", [BS, L], f32)
    iotaRow = din("iotaRow", [128, NG], f32)
    clampR = din("clampR", [128, 1], f32)
    repMc = din("repMc", [128, 128], f32)
    selMfc = din("selMfc", [128, BS], f32)
    maskCc = din("maskCc", [128, 24], f32)
    lmapc = din("lmapc", [24, NG * 128], f32)
    idxc = din("idxc", [128, 2], i32)   # col0: r*16+p%16

    yout = nc.dram_tensor("yout", [B, VS], f32, kind="ExternalOutput").ap()

    RG = [list(range(NC))]

    with tile.TileContext(nc) as tc:
        with tc.tile_pool(name="const", bufs=1) as cp, \
             tc.tile_pool(name="wp", bufs=1) as wp, \
             tc.tile_pool(name="dram", bufs=1, space="DRAM") as dp, \
             tc.tile_pool(name="work", bufs=1) as wk, \
             tc.tile_pool(name="ws", bufs=4) as ws, \
             tc.tile_pool(name="ps_tr", bufs=2, space="PSUM") as ps_tr:

            # ---- 1. tiny constants + CCOM warmup collective ----
            idb = cp.tile([128, 128], bf16)
            nc.sync.dma_start(out=idb[:], in_=identb[:])
            idf = cp.tile([128, 128], f32)
            nc.sync.dma_start(out=idf[:], in_=identf[:])
            gb_sb = cp.tile([1, NL * 512], f32)
            nc.sync.dma_start(out=gb_sb[:], in_=gbias[:])
            ones_f = cp.tile([1, 128], f32)
            nc.vector.memset(ones_f[:], 1.0)
            dum_sb = cp.tile([1, 16], bf16)
            nc.vector.memset(dum_sb[:], 0.0)
            dumi = dp.tile([1, 16], bf16, name="dumi")
            nc.scalar.dma_start(out=dumi[:], in_=dum_sb[:])
            dumo = dp.tile([NC, 16], bf16, name="dumo", addr_space="Shared")
            nc.gpsimd.collective_compute(
                "AllGather", OP.bypass, replica_groups=RG,
                ins=[dumi[:]], outs=[dumo[:]])

            # ---- 2. LSTM dependencies, layer-0 weights first ----
            lstm_scope = [
                tc.tile_pool(name="lstm_in", bufs=1),
                tc.tile_pool(name="lstm_work", bufs=1),
                tc.tile_pool(name="ps_g", bufs=1, space="PSUM"),
            ]
            lw = lstm_scope[0].__enter__()
            lpw = lstm_scope[1].__enter__()
            ps_g = lstm_scope[2].__enter__()
            x0T_sb = lw.tile([128, H], bf16, tag="xT", bufs=2)
            nc.sync.dma_start(out=x0T_sb[:], in_=x0Tr[:])
            h0T_sb = lw.tile([128, NL * 8 * 128], bf16)
            nc.sync.dma_start(out=h0T_sb[:], in_=h0Tr[:])
            c0_sb = lw.tile([B, NL * HS], f32)
            nc.sync.dma_start(out=c0_sb[:], in_=c0s[:])
            w_sb = lw.tile([128, NL * 2 * 8 * 512], bf16)
            for l in range(NL):
                s = l * 2 * 8 * 512
                e = (l + 1) * 2 * 8 * 512
                nc.sync.dma_start(out=w_sb[:, s:e], in_=wTr[:, s:e])

            # ---- 3. attention + o2 constants ----
            iotaL_sb = cp.tile([BS, L], f32)
            nc.sync.dma_start(out=iotaL_sb[:], in_=iotaL[:])
            ab1_sb = cp.tile([1, HALF], f32)
            nc.sync.dma_start(out=ab1_sb[:], in_=ab1r[:])
            iotaR_sb = cp.tile([128, NG], f32)
            nc.sync.dma_start(out=iotaR_sb[:], in_=iotaRow[:])
            clamp_sb = cp.tile([128, 1], f32)
            nc.sync.dma_start(out=clamp_sb[:], in_=clampR[:])
            repM_sb = cp.tile([128, 128], f32)
            nc.sync.dma_start(out=repM_sb[:], in_=repMc[:])
            selMf_sb = cp.tile([128, BS], f32)
            nc.sync.dma_start(out=selMf_sb[:], in_=selMfc[:])
            maskC_sb = cp.tile([128, 24], f32)
            nc.sync.dma_start(out=maskC_sb[:], in_=maskCc[:])
            lmap_sb = cp.tile([24, NG * 128], f32)
            nc.sync.dma_start(out=lmap_sb[:], in_=lmapc[:])
            idxc_sb = cp.tile([128, 2], i32)
            nc.sync.dma_start(out=idxc_sb[:], in_=idxc[:])
            aw2_sb = cp.tile([128, HALF], f32)
            nc.sync.dma_start(out=aw2_sb[:], in_=aW2r[:])
            ab2_sb = cp.tile([128, 1], f32)
            nc.sync.dma_start(out=ab2_sb[:], in_=ab2[:])
            aw1_sb = cp.tile([128, 8 * HALF], bf16)
            nc.sync.dma_start(out=aw1_sb[:], in_=aW1r[:])
            db1_sb = wp.tile([1, HS], f32)
            nc.sync.dma_start(out=db1_sb[:], in_=db1r[:])
            dw1_sb = wp.tile([128, 16 * HS], bf16)
            nc.sync.dma_start(out=dw1_sb[:], in_=dW1r[:])
            db2_sb = wp.tile([1, VS], f32)
            nc.sync.dma_start(out=db2_sb[:], in_=db2r[:])

            # ============================ LSTM ============================
            outT_sb = wk.tile([128, H], bf16)     # final hidden, transposed
            ago3 = dp.tile([H, 256], bf16, name="ago3", addr_space="Shared")
            out_bh = dp.tile([B, H], bf16, name="out_bh")

            g_ps = [ps_g.tile([128, 512], f32, tag=f"g{l}", name=f"g_ps{l}")
                    for l in range(NL)]

            def g_group(l, src, xts, first, last):
                # 8 accumulating matmuls of one source into g_ps[l]
                for k in range(8):
                    lhsT = (xts[:, k * 128:(k + 1) * 128] if src == 0
                            else h0T_sb[:, (l * 8 + k) * 128:
                                        (l * 8 + k + 1) * 128])
                    nc.tensor.matmul(
                        out=g_ps[l][:],
                        lhsT=lhsT,
                        rhs=w_sb[:, ((l * 2 + src) * 8 + k) * 512:
                                 ((l * 2 + src) * 8 + k + 1) * 512],
                        start=(first and k == 0), stop=(last and k == 7))

            def g_bias_h(l):
                # bias + h-side partials for layer l (no x dependency)
                nc.tensor.matmul(out=g_ps[l][:], lhsT=ones_f[:],
                                 rhs=gb_sb[:, l * 512:(l + 1) * 512],
                                 start=True, stop=False)
                g_group(l, 1, None, False, False)

            # layer 0: everything is available up front
            g_bias_h(0)
            xT_sb = x0T_sb
            g_group(0, 0, xT_sb, False, True)

            for l in range(NL):
                # gate rows are host-packed (i, f, o, g): one sigmoid
                # covers i/f/o, one tanh covers g
                sfo = lpw.tile([128, 384], f32, tag="sfo")
                nc.scalar.activation(out=sfo[:], in_=g_ps[l][:, 0:384],
                                     func=AF.Sigmoid)
                g_t = lpw.tile([128, 128], f32, tag="g_t")
                nc.scalar.activation(out=g_t[:], in_=g_ps[l][:, 384:512],
                                     func=AF.Tanh)
                i_s, f_s, o_s = sfo[:, 0:128], sfo[:, 128:256], sfo[:, 256:384]
                cnew = lpw.tile([128, 128], f32, tag="cnew")
                nc.vector.tensor_mul(out=cnew[:], in0=f_s,
                                     in1=c0_sb[:, l * HS:(l + 1) * HS])
                ig = lpw.tile([128, 128], f32, tag="ig")
                nc.gpsimd.tensor_mul(out=ig[:], in0=i_s, in1=g_t[:])
                nc.vector.tensor_add(out=cnew[:], in0=cnew[:], in1=ig[:])
                tc_t = lpw.tile([128, 128], f32, tag="tc_t")
                nc.scalar.activation(out=tc_t[:], in_=cnew[:], func=AF.Tanh)
                xfb = lpw.tile([128, 128], bf16, tag="xfb")
                nc.vector.tensor_mul(out=xfb[:], in0=o_s, in1=tc_t[:])

                if l < NL - 1:
                    tr_ps = ps_g.tile([128, 128], bf16, tag="tr", bufs=2)
                    nc.tensor.transpose(out=tr_ps[:], in_=xfb[:],
                                        identity=idb[:])
                    xTs = lpw.tile([128, 128], bf16, tag="xTs")
                    nc.vector.tensor_copy(out=xTs[:], in_=tr_ps[:])
                    agi = dp.tile([128, 128], bf16, name=f"agi{l}",
                                  tag=f"agi{l}")
                    nc.scalar.dma_start(out=agi[:], in_=xTs[:])
                    ago = dp.tile([H, 128], bf16, name=f"ago{l}",
                                  tag=f"ago{l}", addr_space="Shared")
                    nc.gpsimd.collective_compute(
                        "AllGather", OP.bypass, replica_groups=RG,
                        ins=[agi[:]], outs=[ago[:]])
                    # h+bias partials of the NEXT layer run during the AG
                    g_bias_h(l + 1)
                    xT_sb = lw.tile([128, H], bf16, tag="xT", bufs=2)
                    nc.scalar.dma_start(
                        out=xT_sb[:].rearrange("p (k b) -> p k b", b=128),
                        in_=ago[:].rearrange("(k p) b -> p k b", p=128))
                    g_group(l + 1, 0, xT_sb, False, True)
                else:
                    tr_ps = ps_g.tile([128, 128], bf16, tag="tr", bufs=2)
                    nc.tensor.transpose(out=tr_ps[:], in_=xfb[:],
                                        identity=idb[:])
                    pk = lpw.tile([128, 256], bf16, tag="pk")
                    nc.vector.tensor_copy(out=pk[:, 0:128], in_=tr_ps[:])
                    nc.vector.tensor_copy(out=pk[:, 128:256], in_=xfb[:])
                    agi3 = dp.tile([128, 256], bf16, name="agi3")
                    nc.scalar.dma_start(out=agi3[:], in_=pk[:])
                    nc.gpsimd.collective_compute(
                        "AllGather", OP.bypass, replica_groups=RG,
                        ins=[agi3[:]], outs=[ago3[:]])
                    nc.scalar.dma_start(
                        out=outT_sb[:].rearrange("p (k b) -> p k b", b=128),
                        in_=ago3[:, 0:128].rearrange("(k p) b -> p k b",
                                                     p=128))
                    # reshuffle to out[b, h] layout in DRAM for the
                    # attention score broadcast (local DMA, no core offset)
                    nc.scalar.dma_start(
                        out=out_bh[:].rearrange("b (k f) -> b k f", f=128),
                        in_=AP(ago3[:].tensor, 128,
                               [[256, 128], [128 * 256, 8], [1, 128]]))
            for _cm in reversed(lstm_scope):
                _cm.__exit__(None, None, None)

            # ============================ p-chain ============================
            with tc.tile_pool(name="att", bufs=1) as at:
              with tc.tile_pool(name="ps_a", bufs=1, space="PSUM") as ps_a:
                pt_ps = ps_a.tile([128, HALF], f32, tag="pt")
                for k in range(8):
                    nc.tensor.matmul(out=pt_ps[:],
                                     lhsT=outT_sb[:, k * 128:(k + 1) * 128],
                                     rhs=aw1_sb[:, k * HALF:(k + 1) * HALF],
                                     start=(k == 0), stop=False)
                nc.tensor.matmul(out=pt_ps[:], lhsT=ones_f[:], rhs=ab1_sb[:],
                                 start=False, stop=True)
                pt = at.tile([128, HALF], f32)
                nc.scalar.activation(out=pt[:], in_=pt_ps[:], func=AF.Tanh)
                scr5 = at.tile([128, HALF], f32)
                z = at.tile([128, 1], f32)
                nc.vector.scalar_tensor_tensor(
                    out=scr5[:], in0=pt[:], scalar=1.0,
                    in1=aw2_sb[:], op0=OP.mult, op1=OP.mult,
                    accum_out=z[:])
                sg = at.tile([128, 1], f32)
                nc.scalar.activation(out=sg[:], in_=z[:], func=AF.Sigmoid,
                                     bias=ab2_sb[:])
                r_ = at.tile([128, 1], f32)        # p - W + 0.5
                nc.vector.tensor_scalar(out=r_[:], in0=sg[:],
                                        scalar1=float(S_val), scalar2=0.5,
                                        op0=OP.mult, op1=OP.add)
                # floor(r_) robust to the f32->i32 cast rounding mode:
                # f = cast(r_); if f > r_ then f -= 1
                ti = at.tile([128, 1], i32)
                nc.vector.tensor_copy(out=ti[:], in_=r_[:])
                tf = at.tile([128, 1], f32)
                nc.vector.tensor_copy(out=tf[:], in_=ti[:])
                cond = at.tile([128, 1], f32)
                nc.vector.tensor_tensor(out=cond[:], in0=tf[:], in1=r_[:],
                                        op=OP.is_gt)
                stf = at.tile([128, 1], f32)       # start (rounded, float)
                nc.vector.tensor_sub(out=stf[:], in0=tf[:], in1=cond[:])
                d0 = at.tile([128, 1], f32)        # start - p = stf - r_ + .5 - W
                nc.vector.tensor_sub(out=d0[:], in0=stf[:], in1=r_[:])
                nc.vector.tensor_scalar_add(out=d0[:], in0=d0[:],
                                            scalar1=0.5 - float(W))

                # ---- one replication matmul: rows 0:16 are this core's b ----
                pk2 = at.tile([128, 2], f32)
                nc.vector.tensor_copy(out=pk2[:, 0:1], in_=stf[:])
                nc.vector.tensor_copy(out=pk2[:, 1:2], in_=d0[:])
                rep_ps = ps_a.tile([128, 2], f32, tag="scr", bufs=1)
                nc.tensor.matmul(out=rep_ps[:], lhsT=repM_sb[:],
                                 rhs=pk2[:], start=True, stop=True)
                rep2 = at.tile([128, 2], f32)
                nc.vector.tensor_copy(out=rep2[:], in_=rep_ps[:])
                stf16 = rep2[0:BS, 0:1]
                d016 = rep2[0:BS, 1:2]
                stf_rep = rep2[:, 0:1]
                orep = at.tile([128, H], bf16)
                nc.gpsimd.indirect_dma_start(
                    out=orep[:], out_offset=None, in_=out_bh[:],
                    in_offset=IOA(ap=idxc_sb[:, 0:1], axis=0))

                # ---- fused gather indices for all 3 groups ----
                st16 = at.tile([128, 1], f32)
                nc.vector.tensor_scalar_mul(out=st16[:], in0=stf_rep,
                                            scalar1=float(BS))
                idxf3 = at.tile([128, NG], f32)
                nc.vector.tensor_scalar(out=idxf3[:], in0=iotaR_sb[:],
                                        scalar1=st16[:], scalar2=None,
                                        op0=OP.add)
                nc.vector.tensor_scalar(out=idxf3[:], in0=idxf3[:],
                                        scalar1=clamp_sb[:], scalar2=None,
                                        op0=OP.min)
                idx3 = at.tile([128, NG], i32)
                nc.vector.tensor_copy(out=idx3[:], in_=idxf3[:])
                sel = [at.tile([128, H], bf16, name=f"sel{g}", tag=f"sel{g}")
                       for g in range(NG)]
                cnt = [128, 128, ROWS - 256]
                for g in range(NG):
                    nc.gpsimd.indirect_dma_start(
                        out=sel[g][0:cnt[g], :], out_offset=None,
                        in_=encs[:],
                        in_offset=IOA(ap=idx3[0:cnt[g], g:g + 1], axis=0))

                # ---- window masks + gauss weights (fill the gather wait) ----
                sc_col = at.tile([128, NG], f32)
                nc.vector.memset(sc_col[:], 0.0)
                aa24 = at.tile([BS, 24], f32)
                nc.vector.memset(aa24[:], 0.0)
                pos = at.tile([BS, L], f32)
                nc.vector.tensor_scalar(out=pos[:], in0=iotaL_sb[:],
                                        scalar1=stf16, scalar2=None, op0=OP.add)
                v1 = at.tile([BS, L], f32)
                nc.vector.tensor_scalar(out=v1[:], in0=pos[:], scalar1=float(W),
                                        scalar2=None, op0=OP.is_ge)
                v2 = at.tile([BS, L], f32)
                nc.vector.tensor_scalar(out=v2[:], in0=pos[:],
                                        scalar1=float(S_val + W),
                                        scalar2=None, op0=OP.is_lt)
                nc.vector.tensor_mul(out=v1[:], in0=v1[:], in1=v2[:])
                # gauss: pos - p = l + (start - p) = l + d016
                dd = at.tile([BS, L], f32)
                nc.vector.tensor_scalar(out=dd[:], in0=iotaL_sb[:],
                                        scalar1=d016, scalar2=None, op0=OP.add)
                d2 = at.tile([BS, L], f32)
                nc.vector.tensor_mul(out=d2[:], in0=dd[:], in1=dd[:])
                gs = at.tile([BS, L], f32)
                nc.scalar.activation(out=gs[:], in_=d2[:], func=AF.Exp,
                                     scale=-1.0 / (2.0 * STD2))

                # ---- score: out . sel per (l,b) row ----
                for g in range(NG):
                    scrH = at.tile([128, H], bf16, tag="scrH", bufs=1)
                    nc.vector.scalar_tensor_tensor(
                        out=scrH[0:cnt[g], :], in0=orep[0:cnt[g], :], scalar=1.0,
                        in1=sel[g][0:cnt[g], :], op0=OP.mult, op1=OP.mult,
                        accum_out=sc_col[0:cnt[g], g:g + 1])

                # -------- [16, 21] softmax block --------
                # sc16[b, l] = sc_col[(l%8)*16+b, l//8] via a selector matmul:
                # X[p, c] = sc_col[p, c//8] * maskC[p, c]; sc24 = selMf.T @ X
                X = at.tile([128, 24], f32)
                nc.vector.tensor_tensor(
                    out=X[:].rearrange("p (g li) -> p g li", g=NG),
                    in0=sc_col[:].unsqueeze(2).broadcast_to([128, NG, 8]),
                    in1=maskC_sb[:].rearrange("p (g li) -> p g li", g=NG),
                    op=OP.mult)
                sc_ps = ps_a.tile([BS, 24], f32, tag="scr", bufs=1)
                nc.tensor.matmul(out=sc_ps[:], lhsT=selMf_sb[:], rhs=X[:],
                                 start=True, stop=True)
                sc24 = at.tile([BS, 24], f32)
                nc.vector.tensor_copy(out=sc24[:], in_=sc_ps[:])
                sc16 = sc24[:, 0:L]

                sm = at.tile([BS, L], f32)
                nc.vector.scalar_tensor_tensor(
                    out=sm[:], in0=sc16, scalar=-1e-12,
                    in1=v1[:], op0=OP.add, op1=OP.mult)
                nc.vector.tensor_scalar_add(out=sm[:], in0=sm[:], scalar1=1e-12)
                mx = at.tile([BS, 1], f32)
                nc.vector.tensor_reduce(out=mx[:], in_=sm[:],
                                        axis=mybir.AxisListType.X, op=OP.max)
                nmx = at.tile([BS, 1], f32)
                nc.vector.tensor_scalar_mul(out=nmx[:], in0=mx[:], scalar1=-1.0)
                ex = at.tile([BS, L], f32)
                se = at.tile([BS, 1], f32)
                nc.scalar.activation(out=ex[:], in_=sm[:], func=AF.Exp,
                                     bias=nmx[:], accum_out=se[:])
                ri = at.tile([BS, 1], f32)
                nc.vector.reciprocal(out=ri[:], in_=se[:])
                aa = at.tile([BS, L], f32)
                nc.vector.tensor_scalar(out=aa[:], in0=ex[:], scalar1=ri[:],
                                        scalar2=None, op0=OP.mult)
                nc.vector.tensor_mul(out=aa[:], in0=aa[:], in1=gs[:])
                # relayout a -> a-weighted selector Sa via PE:
                # aaT = aa.T (PE transpose), Sa_g = (Lmap_g.T @ aaT) * selMf
                nc.vector.tensor_copy(out=aa24[:, 0:L], in_=aa[:])
                aaT_ps = ps_a.tile([24, BS], f32, tag="scr", bufs=1)
                nc.tensor.transpose(out=aaT_ps[:], in_=aa24[:],
                                    identity=idf[0:BS, 0:BS])
                aaT = at.tile([24, BS], f32)
                nc.vector.tensor_copy(out=aaT[:], in_=aaT_ps[:])

                # ==================== ctx ====================
                ctx_ps = ps_a.tile([BS, H], f32, tag="ctx")
                for g in range(NG):
                    sa_ps = ps_a.tile([128, BS], f32, tag="sa", bufs=1)
                    nc.tensor.matmul(out=sa_ps[:],
                                     lhsT=lmap_sb[:, g * 128:(g + 1) * 128],
                                     rhs=aaT[:], start=True, stop=True)
                    sa = at.tile([128, BS], bf16, tag="sab", bufs=3)
                    nc.vector.tensor_mul(out=sa[:], in0=sa_ps[:],
                                         in1=selMf_sb[:])
                    for n in range(2):
                        nc.tensor.matmul(
                            out=ctx_ps[:, n * 512:(n + 1) * 512],
                            lhsT=sa[0:cnt[g], :],
                            rhs=sel[g][0:cnt[g], n * 512:(n + 1) * 512],
                            start=(g == 0), stop=(g == NG - 1))
                ctxb = at.tile([BS, H], bf16)
                nc.vector.tensor_copy(out=ctxb[:], in_=ctx_ps[:])
              if True:
                ctxi = dp.tile([BS, H], bf16, name="ctxi")
                nc.scalar.dma_start(out=ctxi[:], in_=ctxb[:])
                ctxo = dp.tile([B, H], bf16, name="ctxo", addr_space="Shared")
                nc.gpsimd.collective_compute(
                    "AllGather", OP.bypass, replica_groups=RG,
                    ins=[ctxi[:]], outs=[ctxo[:]])

                # ==================== o2 (TP over 1024 outputs) ============
                ctx_sb = at.tile([B, H], bf16)
                nc.scalar.dma_start(out=ctx_sb[:], in_=ctxo[:])
                ctxT = at.tile([128, H], bf16)
                for k in range(8):
                    trp = ps_tr.tile([128, 128], bf16, tag="tr2")
                    nc.tensor.transpose(out=trp[:],
                                        in_=ctx_sb[:, k * 128:(k + 1) * 128],
                                        identity=idb[:])
                    nc.vector.tensor_copy(out=ctxT[:, k * 128:(k + 1) * 128],
                                          in_=trp[:])
                with tc.tile_pool(name="ps_o2", bufs=1, space="PSUM") as ps_o2:
                    o2_ps = ps_o2.tile([128, HS], f32, tag="o2")
                    for k in range(16):
                        lhsT = (ctxT[:, k * 128:(k + 1) * 128] if k < 8
                                else outT_sb[:, (k - 8) * 128:(k - 7) * 128])
                        nc.tensor.matmul(
                            out=o2_ps[:], lhsT=lhsT,
                            rhs=dw1_sb[:, k * HS:(k + 1) * HS],
                            start=(k == 0), stop=False)
                    nc.tensor.matmul(out=o2_ps[:], lhsT=ones_f[:],
                                     rhs=db1_sb[:], start=False, stop=True)
                    o2b = at.tile([128, HS], bf16)
                    nc.scalar.activation(out=o2b[:], in_=o2_ps[:],
                                         func=AF.Tanh)
                    trp = ps_tr.tile([128, 128], bf16, tag="tr2")
                    nc.tensor.transpose(out=trp[:], in_=o2b[:], identity=idb[:])
                    o2Ts = at.tile([128, 128], bf16)
                    nc.vector.tensor_copy(out=o2Ts[:], in_=trp[:])
                    agiO = dp.tile([128, 128], bf16, name="agiO")
                    nc.scalar.dma_start(out=agiO[:], in_=o2Ts[:])
                    agoO = dp.tile([H, 128], bf16, name="agoO",
                                   addr_space="Shared")
                    nc.gpsimd.collective_compute(
                        "AllGather", OP.bypass, replica_groups=RG,
                        ins=[agiO[:]], outs=[agoO[:]])
                    o2T = at.tile([128, H], bf16)
                    nc.scalar.dma_start(
                        out=o2T[:].rearrange("p (k b) -> p k b", b=128),
                        in_=agoO[:].rearrange("(k p) b -> p k b", p=128))

                # ==================== vocab ====================
                with tc.tile_pool(name="ps_y", bufs=3, space="PSUM") as ps_y, \
                     tc.tile_pool(name="ysb", bufs=3) as ysb:
                    for n in range(NCH):
                        cw = min(512, VS - n * 512)
                        wtile = ws.tile([128, 8 * 512], bf16, tag="wt")
                        nc.sync.dma_start(
                            out=wtile[:],
                            in_=dW2s[:, n * 8 * 512:(n + 1) * 8 * 512])
                        y_ps = ps_y.tile([128, 512], f32, tag="y")
                        for k in range(8):
                            nc.tensor.matmul(
                                out=y_ps[:, 0:cw],
                                lhsT=o2T[:, k * 128:(k + 1) * 128],
                                rhs=wtile[:, k * 512:k * 512 + cw],
                                start=(k == 0), stop=False)
                        nc.tensor.matmul(
                            out=y_ps[:, 0:cw], lhsT=ones_f[:],
                            rhs=db2_sb[:, n * 512:n * 512 + cw],
                            start=False, stop=True)
                        y_sb = ysb.tile([128, 512], f32, tag="ysb")
                        nc.vector.tensor_copy(out=y_sb[:, 0:cw], in_=y_ps[:, 0:cw])
                        nc.sync.dma_start(out=yout[:, n * 512:n * 512 + cw],
                                          in_=y_sb[:, 0:cw])

    nc.compile()
    return nc


def _sbufize(a, p=128):
    """[m*p, c] -> [p, m*c]: host pre-layout so the device DMA is a
    contiguous 2D copy into a [128, m, c] SBUF tile."""
    m = a.shape[0] // p
    return np.ascontiguousarray(
        a.reshape(m, p, a.shape[1]).transpose(1, 0, 2).reshape(p, -1))


def _prep_inputs(inputs):
    """Host-side layout: returns list of per-core in_maps."""
    import ml_dtypes
    bf16 = ml_dtypes.bfloat16

    enc = np.asarray(inputs["encoder_output"], np.float32)      # [T, B, H]
    h0 = np.asarray(inputs["h0"], np.float32)
    c0 = np.asarray(inputs["c0"], np.float32)
    emb = np.asarray(inputs["emb"], np.float32)
    Wih = np.asarray(inputs["Wih"], np.float32)
    Whh = np.asarray(inputs["Whh"], np.float32)
    bih = np.asarray(inputs["bih"], np.float32)
    bhh = np.asarray(inputs["bhh"], np.float32)
    aW1 = np.asarray(inputs["aW1"], np.float32)
    aW2 = np.asarray(inputs["aW2"], np.float32)
    ab2 = np.asarray(inputs["ab2"], np.float32)
    dW1 = np.asarray(inputs["dW1"], np.float32)
    db1 = np.asarray(inputs["db1"], np.float32)
    dW2 = np.asarray(inputs["dW2"], np.float32)
    db2 = np.asarray(inputs["db2"], np.float32)
    word = np.asarray(inputs["word"]).astype(np.int64)

    x0 = emb[word[0]]                                            # [B, H]
    x0Tr = _sbufize(np.ascontiguousarray(x0.T)).astype(bf16)
    h0Tr = _sbufize(
        np.ascontiguousarray(h0.transpose(0, 2, 1)).reshape(NL * H, B)
    ).astype(bf16)

    ident_b = np.eye(128, dtype=np.float32).astype(bf16)
    ident_f = np.eye(128, dtype=np.float32)
    iotaL = np.tile(np.arange(L, dtype=np.float32).reshape(1, L), (BS, 1))
    iotaRow = np.zeros((128, NG), np.float32)
    for g in range(NG):
        for p in range(128):
            r = g * 128 + p
            iotaRow[p, g] = float(r if r < ROWS else 0)
    clampR = ((T - 1) * BS + (np.arange(128) % BS)).astype(np.float32).reshape(128, 1)
    selMf = np.zeros((128, BS), np.float32)
    for p in range(128):
        selMf[p, p % BS] = 1.0
    maskC = np.zeros((128, 24), np.float32)
    for p in range(128):
        for c in range(24):
            if p // BS == c % 8:
                maskC[p, c] = 1.0
    lmap = np.zeros((24, NG * 128), np.float32)
    for g in range(NG):
        for row in range(128):
            lmap[g * 8 + row // BS, g * 128 + row] = 1.0

    aW1r = _sbufize(np.ascontiguousarray(aW1.T)).astype(bf16)    # [128, 8*512]
    aW2r = np.tile(aW2.reshape(1, HALF), (128, 1)).astype(np.float32)
    ab2r = np.tile(ab2.reshape(1, 1), (128, 1)).astype(np.float32)
    ab1r = np.asarray(inputs["ab1"], np.float32).reshape(1, HALF)

    in_maps = []
    for r in range(NC):
        hs = slice(r * HS, (r + 1) * HS)
        # gate-row packing (i, f, o, g) so one Sigmoid covers i/f/o
        rows = np.concatenate([np.arange(g * H + r * HS, g * H + (r + 1) * HS)
                               for g in (0, 1, 3, 2)])
        wT_l = []
        gb = np.zeros((NL, 512), np.float32)
        for l in range(NL):
            wT_l.append(np.ascontiguousarray(Wih[l][rows, :].T))  # [H, 512]
            wT_l.append(np.ascontiguousarray(Whh[l][rows, :].T))
            gb[l] = bih[l][rows] + bhh[l][rows]
        wTr = _sbufize(np.concatenate(wT_l, axis=0)).astype(bf16)
        c0s = np.ascontiguousarray(
            np.stack([c0[l][:, hs] for l in range(NL)], axis=1).reshape(B, NL * HS))
        bs = slice(r * BS, (r + 1) * BS)
        encs = np.ascontiguousarray(enc[:, bs, :]).reshape(T * BS, H).astype(bf16)
        vs = slice(r * VS, (r + 1) * VS)
        dW2T = np.ascontiguousarray(dW2[vs, :].T)                # [H, VS]
        # (n, k)-tiled stream layout: chunk n is one contiguous [128, 8*512]
        dW2s = np.zeros((128, NCH * 8 * 512), np.float32)
        for n in range(NCH):
            cw = min(512, VS - n * 512)
            blk = dW2T[:, n * 512:n * 512 + cw]                  # [1024, cw]
            blk = blk.reshape(8, 128, cw)
            dW2s[:, n * 8 * 512:(n + 1) * 8 * 512].reshape(
                128, 8, 512)[:, :, 0:cw] = blk.transpose(1, 0, 2)
        dW2s = dW2s.astype(bf16)
        db2r_c = db2[vs].reshape(1, VS)
        dW1r = _sbufize(
            np.ascontiguousarray(dW1[hs, :].T)).astype(bf16)     # [128, 16*128]
        db1r = db1[hs].reshape(1, HS)
        idxcm = np.zeros((128, 2), np.int32)
        idxcm[:, 0] = r * BS + (np.arange(128) % BS)
        idxcm[0:BS, 1] = r * BS + np.arange(BS)
        repM = np.zeros((128, 128), np.float32)
        for m in range(128):
            repM[r * BS + (m % BS), m] = 1.0
        in_maps.append({
            "x0Tr": np.ascontiguousarray(x0Tr),
            "h0Tr": h0Tr, "c0s": c0s, "wTr": wTr,
            "gbias": gb.reshape(1, NL * 512),
            "aW1r": aW1r, "aW2r": aW2r, "ab2": ab2r,
            "dW1r": dW1r, "db1r": db1r, "ab1r": ab1r,
            "dW2s": dW2s, "db2r": db2r_c,
            "encs": encs, "identb": ident_b, "identf": ident_f,
            "iotaL": iotaL, "iotaRow": iotaRow, "clampR": clampR,
            "idxc": idxcm, "repMc": repM,
            "selMfc": selMf, "maskCc": maskC, "lmapc": lmap,
        })
    return in_maps


def kernel(**inputs):
    from concourse import bass_utils
    S_val = float(np.asarray(inputs["S"]))
    key = ("mod", S_val)
    if key not in _CACHE:
        _CACHE[key] = _build(S_val)
    nc = _CACHE[key]
    in_maps = _prep_inputs(inputs)
    res = bass_utils.run_bass_kernel_spmd(nc, in_maps, core_ids=list(range(NC)))
    y = np.concatenate([res.results[r]["yout"] for r in range(NC)], axis=1)
    return y.reshape(1, B, V).astype(np.float32)
